# revision 35
# baseline (speedup 1.0000x reference)
"""Trainium2 Bass kernel for nn_EquivariantBlock (EGNN message-passing block).

Sharding: edges sorted by destination node (row) on the host and sharded by
contiguous node range (N/8 nodes per core) so each core owns every edge of
its node range; per-edge work is edge-parallel, the segment-sum aggregates
are core-local and disjoint, and the two collectives are AllGathers of the
per-core node shards (hh_new, x_out).

Device-side per 512-edge tile:
  - row-side features "gathered" by one-hot matmul from the SBUF-resident
    row block (rows sorted => no DMA),
  - col-side features gathered with [P,1]-offset indirect DMAs from a packed
    DRAM table G1 = [hh | x] (phase 1) / G2 = hh_new (phase 2),
  - edge MLPs run in feature-on-partition (transposed) space on PE,
  - segment sums are one-hot matmuls accumulated in PSUM per 128-node block.
"""

import math
import time
from contextlib import ExitStack

import numpy as np

# ---------------------------------------------------------------- constants
D = 128
N_CORES = 8
NORM_FACTOR = 100.0
NORM_CONST = 1.0
LN_EPS = 1e-5
TILE = 512           # edges per MLP tile
CHUNK = 128          # edges per indirect gather / K<=128 matmul
NCH = TILE // CHUNK  # chunks per tile (4)
NB = 128             # nodes per row block
GROW = D + 8         # G1 row: 128 hh + 3 x + 5 pad (544 B)

_CACHE = {}


# ------------------------------------------------------------------- host prep
def host_prep(x, edge_index, edge_mask, edge_attr, n_nodes):
    """Sort/shard/pad edges; build per-core device input arrays."""
    row = np.asarray(edge_index[0]).astype(np.int64)
    col = np.asarray(edge_index[1]).astype(np.int64)
    emask = np.asarray(edge_mask, np.float32).reshape(-1)
    eattr = np.asarray(edge_attr, np.float32).reshape(-1)

    ns = n_nodes // N_CORES
    n_blocks = math.ceil(ns / NB)
    ns_pad = n_blocks * NB

    x = np.asarray(x, np.float32)
    order = np.argsort(row, kind="stable")
    row_s, col_s = row[order], col[order]
    emask_s, eattr_s = emask[order], eattr[order]

    core_of = row_s // ns
    blk_of = (row_s % ns) // NB
    counts = np.zeros((N_CORES, n_blocks), np.int64)
    np.add.at(counts, (core_of, blk_of), 1)
    tiles_per_block = [
        int(math.ceil(max(1, int(counts[:, b].max())) / TILE))
        for b in range(n_blocks)
    ]
    e_pad = TILE * sum(tiles_per_block)
    n_chunks = e_pad // CHUNK
    n_tiles = e_pad // TILE

    col_idx = np.zeros((N_CORES, n_chunks, CHUNK), np.int32)
    rl = np.zeros((N_CORES, n_chunks, CHUNK), np.float32)
    at = np.zeros((N_CORES, n_chunks, CHUNK), np.float32)
    em = np.zeros((N_CORES, n_chunks, CHUNK), np.float32)
    xr = np.zeros((N_CORES, n_chunks, CHUNK, 3), np.float32)

    starts = np.zeros(N_CORES * n_blocks, np.int64)
    np.cumsum(counts.reshape(-1)[:-1], out=starts[1:])
    starts = starts.reshape(N_CORES, n_blocks)

    for c in range(N_CORES):
        pos = 0
        for b in range(n_blocks):
            s = int(starts[c, b])
            k = int(counts[c, b])
            pe = np.arange(pos, pos + k)
            ch, off = pe // CHUNK, pe % CHUNK
            col_idx[c, ch, off] = col_s[s:s + k]
            rl[c, ch, off] = (row_s[s:s + k] % ns) % NB
            at[c, ch, off] = eattr_s[s:s + k]
            em[c, ch, off] = emask_s[s:s + k]
            xr[c, ch, off, :] = x[row_s[s:s + k]]
            pos += TILE * tiles_per_block[b]
        assert pos == e_pad

    def col_layout(a):
        # [n_chunks, CHUNK] -> [n_tiles, CHUNK, NCH]  (partition-major)
        return np.ascontiguousarray(
            a.reshape(n_tiles, NCH, CHUNK).transpose(0, 2, 1))

    per_core = []
    for c in range(N_CORES):
        x_sh = np.zeros((ns_pad, 3), np.float32)
        x_sh[:ns] = x[c * ns:(c + 1) * ns]
        per_core.append({
            "col_idx": col_layout(col_idx[c]),
            "rl_col": col_layout(rl[c]),
            "em_col": col_layout(em[c]),
            "rlT": np.ascontiguousarray(rl[c].reshape(n_tiles, 1, TILE)),
            "attrT": np.ascontiguousarray(at[c].reshape(n_tiles, 1, TILE)),
            "emT": np.ascontiguousarray(em[c].reshape(n_tiles, 1, TILE)),
            "xr_col": np.ascontiguousarray(
                xr[c].reshape(n_tiles, NCH, CHUNK, 3)
                .transpose(0, 2, 1, 3).reshape(n_tiles, CHUNK, NCH * 3)),
            "x_sh": x_sh,
        })
    meta = dict(n_nodes=n_nodes, ns=ns, ns_pad=ns_pad, n_blocks=n_blocks,
                tiles_per_block=tiles_per_block, e_pad=e_pad,
                n_chunks=n_chunks, n_tiles=n_tiles)
    return per_core, meta


def make_bias_arrays(inp):
    z = np.zeros(D, np.float32)
    cols = np.stack([
        np.asarray(inp.get(k, z), np.float32).reshape(-1) for k in
        ("b_lin", "ba1", "be1", "be2", "bn1", "bn2", "bc1", "bc2")
    ], axis=1)                                    # [128, 8]
    iota_col = np.arange(NB, dtype=np.float32).reshape(NB, 1)
    cols = np.concatenate([cols, iota_col], axis=1)    # [128, 9]
    rows = np.stack([
        np.asarray(inp["ln_g"], np.float32).reshape(-1),
        np.asarray(inp["ln_b"], np.float32).reshape(-1),
        np.full(D, np.float32(np.asarray(inp["ba2"]).reshape(-1)[0])),
        np.arange(D, dtype=np.float32),
    ], axis=0)                                    # [4, 128]
    return cols, rows



# --------------------------------------------------------------- tile patches
# This container's walrus build rejects instructions carrying more than one
# semaphore wait ("Too many sync wait commands").  Redistribute excess waits
# onto single-wait InstNoOp carriers placed just before each instruction on
# the same engine (engine queues are FIFO, so gating is preserved).
_MAX_WAITS = 1
_carrier_n = [0]
_patched = [False]


def _make_carrier(mybir, engine, waits):
    nop = mybir.InstNoOp(name=f"waitcarrier_{_carrier_n[0]}", ins=[], outs=[])
    _carrier_n[0] += 1
    nop.engine = engine
    nop.sync_info = mybir.SyncInfo(on_wait=list(waits), on_update=[])
    return nop


def apply_tile_patch():
    if _patched[0]:
        return
    _patched[0] = True
    import concourse.tile as tile
    import concourse.mybir as mybir

    _orig_lower = tile.TileContext._lower_ordered_insts

    def _patched_lower(self, ordered):
        for bb_name, insts in ordered.items():
            out = []
            for inst in insts:
                si = inst.sync_info
                waits = list(si.on_wait) if si is not None and si.on_wait else []
                if len(waits) > _MAX_WAITS:
                    extra, keep = waits[:-_MAX_WAITS], waits[-_MAX_WAITS:]
                    for k in range(0, len(extra), _MAX_WAITS):
                        out.append(_make_carrier(mybir, inst.engine,
                                                 extra[k:k + _MAX_WAITS]))
                    si.on_wait = keep
                out.append(inst)
            ordered[bb_name] = out
        return _orig_lower(self, ordered)

    def _patched_drain_and_barrier(self, tick_clock, wait_clock):
        from concourse.tile import ScopedClock

        nc = self.nc
        assert self.sems is not None
        allocated = list(self.sems.allocated().values())
        carriers = []
        if allocated:
            for _ in range(48):
                carriers.append(nc.sync.wait_ge(allocated[0], 0))
        drain_inst = nc.sync.drain()
        wait_clock.add_sem_waits(
            drain_inst.ins, ScopedClock({None: tick_clock.global_clock}))
        si = drain_inst.ins.sync_info
        waits = list(si.on_wait) if si and si.on_wait else []
        if len(waits) > 1 and carriers:
            assert len(waits) <= 48, f"need more carriers: {len(waits)}"
            for c, w in zip(carriers, waits[:-1]):
                c.ins.sync_info.on_wait = [w]
            si.on_wait = [waits[-1]]
        nc.all_engine_barrier()
        popped = nc._tile_sem_poison_stack.pop()
        assert popped is self._sem_poison
        nc.clear_and_free_semaphores(allocated)
        nc.all_engine_barrier()

    tile.TileContext._lower_ordered_insts = _patched_lower
    tile.TileContext._drain_and_barrier = _patched_drain_and_barrier


# ------------------------------------------------------------------ bass build
def build_nc(meta, collectives=True):
    import concourse.bass as bass
    import concourse.tile as tile
    from concourse import mybir
    from concourse.masks import make_identity

    apply_tile_patch()

    AF = mybir.ActivationFunctionType
    OP = mybir.AluOpType
    f32 = mybir.dt.float32

    n_nodes = meta["n_nodes"]
    ns = meta["ns"]
    ns_pad = meta["ns_pad"]
    n_blocks = meta["n_blocks"]
    tiles_per_block = meta["tiles_per_block"]
    n_chunks = meta["n_chunks"]
    n_tiles = meta["n_tiles"]
    n_node_tiles = math.ceil(n_nodes / 128)

    nc = bass.Bass("TRN2", target_bir_lowering=False, debug=False,
                   num_devices=N_CORES)

    di = lambda name, shape, dt=f32: nc.dram_tensor(name, shape, dt,
                                                    kind="ExternalInput")
    h_in = di("h_full", [n_nodes, D])
    x_in = di("x_full", [n_nodes, 3])
    h_sh_in = di("h_sh", [ns_pad, D])
    W_lin = di("W_lin", [D, D])
    Wa1 = di("Wa1", [2 * D + 3, D])
    Wa2 = di("Wa2", [D, 1])
    We1 = di("We1", [D + 3, D])
    We2 = di("We2", [D, D])
    Wn1 = di("Wn1", [D, D])
    Wn2 = di("Wn2", [D, D])
    Wc1 = di("Wc1", [2 * D + 3, D])
    Wc2 = di("Wc2", [D, D])
    Wc3 = di("Wc3", [D, 1])
    bcol_in = di("bias_cols", [D, 9])
    brow_in = di("bias_rows", [4, D])
    colx_in = di("col_idx", [n_tiles, CHUNK, NCH], mybir.dt.int32)
    rlc_in = di("rl_col", [n_tiles, CHUNK, NCH])
    emc_in = di("em_col", [n_tiles, CHUNK, NCH])
    rlT_in = di("rlT", [n_tiles, 1, TILE])
    attrT_in = di("attrT", [n_tiles, 1, TILE])
    emT_in = di("emT", [n_tiles, 1, TILE])
    xrc_in = di("xr_col", [n_tiles, CHUNK, NCH * 3])
    xsh_in = di("x_sh", [ns_pad, 3])
    nmask_in = di("nmask_sh", [ns, 1])

    out_h = nc.dram_tensor("out_h", [n_nodes, D], f32, kind="ExternalOutput")
    out_x = nc.dram_tensor("out_x", [n_nodes, 3], f32, kind="ExternalOutput")

    G1 = nc.dram_tensor("G1", [n_nodes, GROW], f32)
    ag_in = nc.dram_tensor("ag_in", [ns, D], f32)
    G2 = nc.dram_tensor("G2", [n_nodes, D], f32, addr_space="Shared")
    agx_in = nc.dram_tensor("agx_in", [ns, 3], f32)
    G2x = nc.dram_tensor("G2x", [n_nodes, 3], f32, addr_space="Shared")
    e3_dram = nc.dram_tensor("e3_dram", [n_tiles, 3, TILE], f32)
    dbg_agg = nc.dram_tensor("dbg_agg", [n_blocks, 128, 128], f32)

    with tile.TileContext(nc) as tc, ExitStack() as ctx:
        singles = ctx.enter_context(tc.tile_pool(name="singles", bufs=1))
        persist = ctx.enter_context(tc.tile_pool(name="persist", bufs=1))
        ep = ctx.enter_context(tc.tile_pool(name="ep", bufs=4))
        epc = ctx.enter_context(tc.tile_pool(name="epc", bufs=2))
        gdst = ctx.enter_context(tc.tile_pool(name="gdst", bufs=6))
        idxt = ctx.enter_context(tc.tile_pool(name="idxt", bufs=6))
        npo = ctx.enter_context(tc.tile_pool(name="npo", bufs=2))
        # PSUM: exactly 8 banks
        ps_gat = ctx.enter_context(tc.tile_pool(name="ps_gat", bufs=1,
                                                space="PSUM"))   # 3 banks
        ps_mlp = ctx.enter_context(tc.tile_pool(name="ps_mlp", bufs=1,
                                                space="PSUM"))   # 2 banks
        ps_sml = ctx.enter_context(tc.tile_pool(name="ps_sml", bufs=1,
                                                space="PSUM"))   # 2 banks
        ps_agg = ctx.enter_context(tc.tile_pool(name="ps_agg", bufs=1,
                                                space="PSUM"))   # 1 bank

        def GAT_HR():
            return ps_gat.tile([128, TILE], f32, tag="gat_hr", name="gat_hr")

        def GAT_HC():
            return ps_gat.tile([128, TILE], f32, tag="gat_hc", name="gat_hc")

        def GAT_MSGT():
            return ps_gat.tile([128, TILE], f32, tag="gat_msgT", name="gat_msgT")

        def MLP_A():
            return ps_mlp.tile([128, TILE], f32, tag="mlp_A", name="mlp_A",
                               bufs=2)

        def MLP_B():
            return ps_mlp.tile([128, TILE], f32, tag="mlp_B", name="mlp_B")

        def SML_ROW():
            return ps_sml.tile([1, TILE], f32, tag="sml_row", name="sml_row")

        # ---------------- weights / constants ----------------
        _wn = [0]

        def wtile(ap, shape):
            nm = f"wt{_wn[0]}"
            _wn[0] += 1
            t = singles.tile(shape, f32, name=nm, tag=nm)
            nc.sync.dma_start(out=t[:], in_=ap[:])
            return t

        w_lin = wtile(W_lin, [D, D])
        wa1a = wtile(Wa1[0:D, :], [D, D])
        wa1b = wtile(Wa1[D:2 * D, :], [D, D])
        wa1c = wtile(Wa1[2 * D:2 * D + 3, :], [3, D])
        wa2 = wtile(Wa2, [D, 1])
        we1a = wtile(We1[0:D, :], [D, D])
        we1b = wtile(We1[D:D + 3, :], [3, D])
        we2 = wtile(We2, [D, D])
        wn1 = wtile(Wn1, [D, D])
        wn2 = wtile(Wn2, [D, D])
        wc1a = wtile(Wc1[0:D, :], [D, D])
        wc1b = wtile(Wc1[D:2 * D, :], [D, D])
        wc1c = wtile(Wc1[2 * D:2 * D + 3, :], [3, D])
        wc2 = wtile(Wc2, [D, D])
        wc3 = wtile(Wc3, [D, 1])
        bc = wtile(bcol_in, [D, 9])
        b_lin, ba1, be1, be2 = bc[:, 0:1], bc[:, 1:2], bc[:, 2:3], bc[:, 3:4]
        bn1, bn2, bc1, bc2 = bc[:, 4:5], bc[:, 5:6], bc[:, 6:7], bc[:, 7:8]
        iota_col = bc[:, 8:9]
        ln_g = wtile(brow_in[0:1, :], [1, D])
        ln_b = wtile(brow_in[1:2, :], [1, D])
        ba2_row = wtile(brow_in[2:3, :], [1, D])
        iota_row = wtile(brow_in[3:4, :], [1, D])
        ba2_s = ba2_row[0:1, 0:1]

        identity = singles.tile([128, 128], f32)
        make_identity(nc, identity[:])
        ones_col = singles.tile([128, 1], f32)
        nc.vector.memset(ones_col[:], 1.0)
        ones_row = singles.tile([1, 128], f32)
        nc.vector.memset(ones_row[:], 1.0)
        eps_col = singles.tile([128, 1], f32)
        nc.vector.memset(eps_col[:], LN_EPS)
        eps8_col = singles.tile([128, 1], f32)
        nc.vector.memset(eps8_col[:], 1e-8)

        # broadcast constants: iota / ln_g / ln_b replicated to all partitions
        iota_bc = singles.tile([128, 128], f32)
        lng_bc = singles.tile([128, 128], f32)
        lnb_bc = singles.tile([128, 128], f32)
        bc_ps = GAT_HR()
        nc.tensor.matmul(bc_ps[:, 0:128], ones_row[:], iota_row[:],
                         start=True, stop=True)
        nc.vector.tensor_copy(iota_bc[:], bc_ps[:, 0:128])
        bc_ps2 = GAT_HC()
        nc.tensor.matmul(bc_ps2[:, 0:128], ones_row[:], ln_g[:],
                         start=True, stop=True)
        nc.vector.tensor_copy(lng_bc[:], bc_ps2[:, 0:128])
        bc_ps3 = GAT_MSGT()
        nc.tensor.matmul(bc_ps3[:, 0:128], ones_row[:], ln_b[:],
                         start=True, stop=True)
        nc.vector.tensor_copy(lnb_bc[:], bc_ps3[:, 0:128])

        # persisted per-edge / per-node data
        cd_sb = persist.tile([128, n_chunks, 4], f32)            # coord_diff
        hhn_sb = persist.tile([128, n_blocks, 128], f32)         # hh_new rows
        hh_own = persist.tile([128, n_blocks, 128], f32)         # hh own rows
        x_blk = persist.tile([128, n_blocks, 4], f32)            # x own rows
        agg_sb = persist.tile([128, n_blocks, 128], f32)         # aggT

        # ============ prologue A: hh = h @ W_lin + b_lin -> G1 ============
        for i in range(n_node_tiles):
            p = min(128, n_nodes - i * 128)
            ht = ep.tile([128, D], f32, tag="ht")
            nc.sync.dma_start(out=ht[:p], in_=h_in[i * 128:i * 128 + p, :])
            hT_ps = GAT_HR()
            nc.tensor.transpose(out=hT_ps[:128, :p], in_=ht[:p, :],
                                identity=identity[:p, :p])
            hT = epc.tile([128, 128], f32, tag="w_hT")
            nc.scalar.activation(hT[:, :p], hT_ps[:, :p], AF.Copy)
            hhT_ps = MLP_A()
            nc.tensor.matmul(hhT_ps[:, :p], w_lin[:], hT[:, :p],
                             start=True, stop=True)
            hhT = epc.tile([128, 128], f32, tag="w_hhT")
            nc.scalar.activation(hhT[:, :p], hhT_ps[:, :p], AF.Identity,
                                 bias=b_lin)
            hh_ps = GAT_HC()
            nc.tensor.transpose(out=hh_ps[:p, 0:128], in_=hhT[:, :p],
                                identity=identity[:])
            gt = epc.tile([128, GROW], f32, tag="w_gt")
            nc.vector.tensor_copy(gt[:p, 0:D], hh_ps[:p, 0:128])
            xt = ep.tile([128, 4], f32, tag="xt")
            nc.sync.dma_start(out=xt[:p, 0:3], in_=x_in[i * 128:i * 128 + p, :])
            nc.vector.tensor_copy(gt[:p, D:D + 3], xt[:p, 0:3])
            nc.vector.memset(gt[:p, D + 3:GROW], 0.0)
            nc.sync.dma_start(out=G1[i * 128:i * 128 + p, :], in_=gt[:p, :])

        # ============ prologue B: hh/x for own row blocks (SBUF) ============
        for b in range(n_blocks):
            hsb = ep.tile([128, D], f32, tag="ht")
            nc.sync.dma_start(out=hsb[:], in_=h_sh_in[b * NB:(b + 1) * NB, :])
            hT_ps = GAT_HR()
            nc.tensor.transpose(out=hT_ps[:, 0:128], in_=hsb[:],
                                identity=identity[:])
            hT = epc.tile([128, 128], f32, tag="w_hT")
            nc.scalar.activation(hT[:], hT_ps[:, 0:128], AF.Copy)
            hhT_ps = MLP_A()
            nc.tensor.matmul(hhT_ps[:, 0:128], w_lin[:], hT[:],
                             start=True, stop=True)
            hhT = epc.tile([128, 128], f32, tag="w_hhT")
            nc.scalar.activation(hhT[:], hhT_ps[:, 0:128], AF.Identity,
                                 bias=b_lin)
            hh_ps = GAT_HC()
            nc.tensor.transpose(out=hh_ps[:, 0:128], in_=hhT[:],
                                identity=identity[:])
            nc.vector.tensor_copy(hh_own[:, b, :], hh_ps[:, 0:128])
            xbt = ep.tile([128, 4], f32, tag="xt")
            nc.sync.dma_start(out=xbt[:, 0:3],
                              in_=xsh_in[b * NB:(b + 1) * NB, :])
            nc.vector.tensor_copy(x_blk[:, b, 0:3], xbt[:, 0:3])

        # ================= phase 1: GCLayer edge pass =================
        def edge_tile_phase1(t, b, start, stop, agg_ps):
            c0 = t * NCH
            rlT_t = ep.tile([1, TILE], f32, tag="rlT")
            nc.sync.dma_start(out=rlT_t[:], in_=rlT_in[t])
            rlc_t = ep.tile([128, NCH], f32, tag="rlc")
            nc.sync.dma_start(out=rlc_t[:], in_=rlc_in[t])
            emc_t = ep.tile([128, NCH], f32, tag="emc")
            nc.sync.dma_start(out=emc_t[:], in_=emc_in[t])
            idx_t = idxt.tile([128, NCH], mybir.dt.int32, tag="idx")
            nc.sync.dma_start(out=idx_t[:], in_=colx_in[t])

            graw = gdst.tile([128, NCH, GROW], f32, tag="graw")
            for c in range(NCH):
                nc.gpsimd.indirect_dma_start(
                    out=graw[:, c, :], out_offset=None, in_=G1[:],
                    in_offset=bass.IndirectOffsetOnAxis(
                        ap=idx_t[:, c:c + 1], axis=0),
                )

            rlbc_sb = epc.tile([128, TILE], f32, tag="rlbc", name="rlbc_sb")
            rl_bcast_ap = bass.AP(
                tensor=rlT_in[t].tensor, offset=rlT_in[t].offset,
                ap=[[0, 128], [1, TILE]])
            nc.sync.dma_start(out=rlbc_sb[:], in_=rl_bcast_ap)
            onehotT = epc.tile([128, TILE], f32, tag="onehotT")
            nc.vector.tensor_scalar(
                onehotT[:], rlbc_sb[:], iota_col, None, OP.is_equal)

            hrT_ps = GAT_HR()
            nc.tensor.matmul(hrT_ps[:], hh_own[:, b, :], onehotT[:],
                             start=True, stop=True)
            hrT = epc.tile([128, TILE], f32, tag="hrT")
            nc.scalar.activation(hrT[:], hrT_ps[:], AF.Copy)

            hcT_ps = GAT_HC()
            for c in range(NCH):
                nc.tensor.transpose(out=hcT_ps[:, c * 128:(c + 1) * 128],
                                    in_=graw[:, c, 0:D], identity=identity[:])
            hcT = epc.tile([128, TILE], f32, tag="hcT")
            nc.vector.tensor_copy(hcT[:], hcT_ps[:])

            dT = epc.tile([128, TILE], f32, tag="dT")
            nc.vector.tensor_sub(dT[:], hcT[:], hrT[:])
            sqT = epc.tile([128, TILE], f32, tag="sqT")
            nc.scalar.activation(sqT[:], dT[:], AF.Square)

            geo2_ps = SML_ROW()
            nc.tensor.matmul(geo2_ps[:], ones_col[:], sqT[:],
                             start=True, stop=True)
            e3 = epc.tile([4, TILE], f32, tag="e3cur", name="e3cur")
            georow = ep.tile([1, TILE], f32, tag="georow")
            nc.scalar.activation(georow[:], geo2_ps[:], AF.Sqrt,
                                 bias=eps8_col[0:1, :])
            nc.sync.dma_start(out=e3[2:3, :], in_=georow[:])
            nc.sync.dma_start(out=e3[1:2, :], in_=attrT_in[t])

            # coord path in [e, 3] layout, per chunk (xr host-gathered)
            xrc_t = ep.tile([128, NCH * 3], f32, tag="xrc")
            nc.sync.dma_start(out=xrc_t[:], in_=xrc_in[t])
            dist_ps = SML_ROW()
            dist_c = ep.tile([128, NCH], f32, tag="dist")
            for c in range(NCH):
                diff = ep.tile([128, NCH, 3], f32, tag="diff")
                nc.vector.tensor_sub(diff[:, c, :], xrc_t[:, c * 3:c * 3 + 3],
                                     graw[:, c, D:D + 3])
                sqd = ep.tile([128, NCH, 3], f32, tag="sqd")
                rad = ep.tile([128, NCH], f32, tag="rad")
                nc.scalar.activation(sqd[:, c, :], diff[:, c, :], AF.Square,
                                     accum_out=rad[:, c:c + 1])
                nc.scalar.activation(dist_c[:, c:c + 1], rad[:, c:c + 1],
                                     AF.Sqrt, bias=eps8_col[:])
                den = ep.tile([128, NCH], f32, tag="den")
                nc.vector.tensor_scalar_add(den[:, c:c + 1], dist_c[:, c:c + 1],
                                            NORM_CONST)
                rec = ep.tile([128, NCH], f32, tag="rec")
                nc.vector.reciprocal(rec[:, c:c + 1], den[:, c:c + 1])
                nc.vector.tensor_scalar_mul(cd_sb[:, c0 + c, 0:3],
                                            diff[:, c, :], rec[:, c:c + 1])
                nc.tensor.transpose(out=dist_ps[:, c * 128:(c + 1) * 128],
                                    in_=dist_c[:, c:c + 1],
                                    identity=identity[:])
            nc.vector.tensor_copy(e3[0:1, :], dist_ps[:])
            nc.sync.dma_start(out=e3_dram[t], in_=e3[0:3, :])

            att1_ps = MLP_A()
            nc.tensor.matmul(att1_ps[:], wa1a[:], hrT[:], start=True, stop=False)
            nc.tensor.matmul(att1_ps[:], wa1b[:], hcT[:], start=False,
                             stop=False)
            nc.tensor.matmul(att1_ps[:], wa1c[:], e3[0:3, :], start=False,
                             stop=True)
            satt1 = epc.tile([128, TILE], f32, tag="satt1")
            nc.scalar.activation(satt1[:], att1_ps[:], AF.Silu, bias=ba1)
            att2_ps = SML_ROW()
            nc.tensor.matmul(att2_ps[:], wa2[:], satt1[:],
                             start=True, stop=True)
            attT = ep.tile([1, TILE], f32, tag="attT")
            nc.scalar.activation(attT[:], att2_ps[:], AF.Sigmoid,
                                 bias=ba2_s)

            msg1_ps = MLP_B()
            nc.tensor.matmul(msg1_ps[:], we1a[:], dT[:], start=True, stop=False)
            nc.tensor.matmul(msg1_ps[:], we1b[:], e3[0:3, :], start=False,
                             stop=True)
            smsg1 = epc.tile([128, TILE], f32, tag="smsg1")
            nc.scalar.activation(smsg1[:], msg1_ps[:], AF.Silu, bias=be1)
            msg2_ps = MLP_B()
            nc.tensor.matmul(msg2_ps[:], we2[:], smsg1[:], start=True,
                             stop=True)
            msgb = epc.tile([128, TILE], f32, tag="msgb")
            nc.scalar.activation(msgb[:], msg2_ps[:], AF.Identity, bias=be2)
            # fold att * edge_mask into msg in transposed space
            emT_t = ep.tile([1, TILE], f32, tag="emT")
            nc.sync.dma_start(out=emT_t[:], in_=emT_in[t])
            attm = ep.tile([1, TILE], f32, tag="attm")
            nc.vector.tensor_tensor(attm[:], attT[:], emT_t[:], OP.mult)
            attbc_ps = GAT_MSGT()
            nc.tensor.matmul(attbc_ps[:], ones_row[:], attm[:],
                             start=True, stop=True)
            msgs = epc.tile([128, TILE], f32, tag="msgs")
            nc.vector.tensor_tensor(msgs[:], msgb[:], attbc_ps[:], OP.mult)

            msgT_ps = GAT_MSGT()
            for c in range(NCH):
                nc.tensor.transpose(out=msgT_ps[:, c * 128:(c + 1) * 128],
                                    in_=msgs[:, c * 128:(c + 1) * 128],
                                    identity=identity[:])
                msg_sb = epc.tile([128, NCH, 128], f32, tag="msg_sb")
                nc.vector.tensor_copy(msg_sb[:, c, :],
                                      msgT_ps[:, c * 128:(c + 1) * 128])
                onehot = epc.tile([128, NCH, 128], f32, tag="onehot")
                nc.vector.tensor_scalar(
                    onehot[:, c, :], iota_bc[:],
                    rlc_t[:, c:c + 1], None, OP.is_equal)
                nc.tensor.matmul(agg_ps[:], msg_sb[:, c, :], onehot[:, c, :],
                                 start=(start and c == 0),
                                 stop=(stop and c == NCH - 1))

        t = 0
        for b in range(n_blocks):
            agg_ps = ps_agg.tile([128, 128], f32, tag="agg")
            for k in range(tiles_per_block[b]):
                edge_tile_phase1(t, b, start=(k == 0),
                                 stop=(k == tiles_per_block[b] - 1),
                                 agg_ps=agg_ps)
                t += 1
            nc.vector.tensor_copy(agg_sb[:, b, :], agg_ps[:])
            nc.sync.dma_start(out=dbg_agg[b], in_=agg_sb[:, b, :])
        assert t == n_tiles

        # ================= node step: MLP + LN + silu, AllGather ==========
        for b in range(n_blocks):
            nvalid = min(NB, ns - b * NB)
            z1_ps = MLP_A()
            nc.tensor.matmul(z1_ps[:, 0:128], wn1[:], agg_sb[:, b, :],
                             start=True, stop=True)
            sz1 = npo.tile([128, 128], f32, tag="sz1")
            nc.scalar.activation(sz1[:], z1_ps[:, 0:128], AF.Silu, bias=bn1)
            z2_ps = MLP_B()
            nc.tensor.matmul(z2_ps[:, 0:128], wn2[:], sz1[:], start=True,
                             stop=True)
            z2T = npo.tile([128, 128], f32, tag="z2T")
            nc.scalar.activation(z2T[:], z2_ps[:, 0:128], AF.Identity,
                                 bias=bn2)
            z2n_ps = GAT_HR()
            nc.tensor.transpose(out=z2n_ps[:, 0:128], in_=z2T[:],
                                identity=identity[:])
            s = npo.tile([128, 128], f32, tag="s")
            nc.vector.tensor_add(s[:], z2n_ps[:, 0:128], hh_own[:, b, :])
            stats = npo.tile([128, 6], f32, tag="stats")
            nc.vector.bn_stats(out=stats[:], in_=s[:])
            mv = npo.tile([128, 2], f32, tag="mv")
            nc.vector.bn_aggr(out=mv[:], in_=stats[:])
            sd = npo.tile([128, 1], f32, tag="sd")
            nc.scalar.activation(sd[:], mv[:, 1:2], AF.Sqrt, bias=eps_col)
            rstd = npo.tile([128, 1], f32, tag="rstd")
            nc.vector.reciprocal(rstd[:], sd[:])
            y = npo.tile([128, 128], f32, tag="y")
            nc.vector.tensor_scalar(y[:], s[:], mv[:, 0:1], rstd[:],
                                    OP.subtract, OP.mult)
            yg = npo.tile([128, 128], f32, tag="yg")
            nc.vector.tensor_tensor(yg[:], y[:], lng_bc[:], OP.mult)
            yb = npo.tile([128, 128], f32, tag="yb")
            nc.vector.tensor_tensor(yb[:], yg[:], lnb_bc[:], OP.add)
            nc.scalar.activation(hhn_sb[:, b, :], yb[:], AF.Silu)
            nc.sync.dma_start(out=ag_in[b * NB:b * NB + nvalid, :],
                              in_=hhn_sb[:nvalid, b, :])

        if collectives:
            with tc.tile_critical():
                cc1 = nc.alloc_semaphore("cc1")
                nc.gpsimd.collective_compute(
                    "AllGather", mybir.AluOpType.bypass,
                    ins=[ag_in[:]], outs=[G2[:]],
                    replica_groups=[list(range(N_CORES))],
                ).then_inc(cc1, 1)
                nc.gpsimd.wait_ge(cc1, 1)
        else:
            for bb in range(n_blocks):
                nv = min(NB, ns - bb * NB)
                tmpg = npo.tile([128, D], f32, tag="sz1", name="tmpg")
                nc.sync.dma_start(out=tmpg[:nv], in_=ag_in[bb * NB:bb * NB + nv, :])
                nc.sync.dma_start(out=G2[bb * NB:bb * NB + nv, :],
                                  in_=tmpg[:nv])

        for i in range(n_node_tiles):
            p = min(128, n_nodes - i * 128)
            tcp = ep.tile([128, D], f32, tag="ht")
            nc.sync.dma_start(out=tcp[:p], in_=G2[i * 128:i * 128 + p, :])
            nc.sync.dma_start(out=out_h[i * 128:i * 128 + p, :], in_=tcp[:p])

        # ================= phase 2: coord MLP edge pass ===================
        def edge_tile_phase2(t, b, start, stop, agx_ps):
            c0 = t * NCH
            rlT_t = ep.tile([1, TILE], f32, tag="rlT")
            nc.sync.dma_start(out=rlT_t[:], in_=rlT_in[t])
            rlc_t = ep.tile([128, NCH], f32, tag="rlc")
            nc.sync.dma_start(out=rlc_t[:], in_=rlc_in[t])
            emc_t = ep.tile([128, NCH], f32, tag="emc")
            nc.sync.dma_start(out=emc_t[:], in_=emc_in[t])
            idx_t = idxt.tile([128, NCH], mybir.dt.int32, tag="idx")
            nc.sync.dma_start(out=idx_t[:], in_=colx_in[t])

            graw = gdst.tile([128, NCH, D], f32, tag="graw2")
            for c in range(NCH):
                nc.gpsimd.indirect_dma_start(
                    out=graw[:, c, :], out_offset=None, in_=G2[:],
                    in_offset=bass.IndirectOffsetOnAxis(
                        ap=idx_t[:, c:c + 1], axis=0),
                )

            rlbc_sb = epc.tile([128, TILE], f32, tag="rlbc", name="rlbc_sb")
            rl_bcast_ap = bass.AP(
                tensor=rlT_in[t].tensor, offset=rlT_in[t].offset,
                ap=[[0, 128], [1, TILE]])
            nc.sync.dma_start(out=rlbc_sb[:], in_=rl_bcast_ap)
            onehotT = epc.tile([128, TILE], f32, tag="onehotT")
            nc.vector.tensor_scalar(
                onehotT[:], rlbc_sb[:], iota_col, None, OP.is_equal)

            grT_ps = GAT_HR()
            nc.tensor.matmul(grT_ps[:], hhn_sb[:, b, :], onehotT[:],
                             start=True, stop=True)
            grT = epc.tile([128, TILE], f32, tag="hrT")
            nc.scalar.activation(grT[:], grT_ps[:], AF.Copy)

            gcT_ps = GAT_HC()
            for c in range(NCH):
                nc.tensor.transpose(out=gcT_ps[:, c * 128:(c + 1) * 128],
                                    in_=graw[:, c, :], identity=identity[:])
            gcT = epc.tile([128, TILE], f32, tag="hcT")
            nc.vector.tensor_copy(gcT[:], gcT_ps[:])

            e3 = epc.tile([4, TILE], f32, tag="e3cur", name="e3cur")
            nc.sync.dma_start(out=e3[0:3, :], in_=e3_dram[t])
            m1_ps = MLP_A()
            nc.tensor.matmul(m1_ps[:], wc1a[:], grT[:], start=True, stop=False)
            nc.tensor.matmul(m1_ps[:], wc1b[:], gcT[:], start=False,
                             stop=False)
            nc.tensor.matmul(m1_ps[:], wc1c[:], e3[0:3, :], start=False,
                             stop=True)
            sm1 = epc.tile([128, TILE], f32, tag="satt1")
            nc.scalar.activation(sm1[:], m1_ps[:], AF.Silu, bias=bc1)
            m2_ps = MLP_B()
            nc.tensor.matmul(m2_ps[:], wc2[:], sm1[:], start=True, stop=True)
            sm2 = epc.tile([128, TILE], f32, tag="smsg1")
            nc.scalar.activation(sm2[:], m2_ps[:], AF.Silu, bias=bc2)
            mT_ps = SML_ROW()
            nc.tensor.matmul(mT_ps[:], wc3[:], sm2[:], start=True, stop=True)
            mrow = ep.tile([1, TILE], f32, tag="attT")
            nc.vector.tensor_copy(mrow[:], mT_ps[:])

            mc_ps = ps_gat.tile([128, TILE], f32, tag="gat_msgT",
                                name="mc_ps")
            for c in range(NCH):
                nc.tensor.transpose(out=mc_ps[:, c * 4:c * 4 + 1],
                                    in_=mrow[:, c * 128:(c + 1) * 128],
                                    identity=identity[0:1, 0:1])
                fac = ep.tile([128, NCH], f32, tag="fac")
                nc.vector.tensor_tensor(fac[:, c:c + 1],
                                        mc_ps[:, c * 4:c * 4 + 1],
                                        emc_t[:, c:c + 1], OP.mult)
                trans = epc.tile([128, NCH, 4], f32, tag="trans")
                nc.vector.tensor_scalar_mul(trans[:, c, 0:3],
                                            cd_sb[:, c0 + c, 0:3],
                                            fac[:, c:c + 1])
                onehot = epc.tile([128, NCH, 128], f32, tag="onehot")
                nc.vector.tensor_scalar(
                    onehot[:, c, :], iota_bc[:],
                    rlc_t[:, c:c + 1], None, OP.is_equal)
                nc.tensor.matmul(agx_ps[0:3, :], trans[:, c, 0:3],
                                 onehot[:, c, :],
                                 start=(start and c == 0),
                                 stop=(stop and c == NCH - 1))

        t = 0
        for b in range(n_blocks):
            nvalid = min(NB, ns - b * NB)
            agx_ps = ps_agg.tile([128, 128], f32, tag="agg")
            for k in range(tiles_per_block[b]):
                edge_tile_phase2(t, b, start=(k == 0),
                                 stop=(k == tiles_per_block[b] - 1),
                                 agx_ps=agx_ps)
                t += 1
            agx_sb = npo.tile([4, 128], f32, tag="agx_sb")
            nc.vector.tensor_copy(agx_sb[0:3, :], agx_ps[0:3, :])
            agxT_ps = GAT_HR()
            nc.tensor.transpose(out=agxT_ps[:, 0:3], in_=agx_sb[0:3, :],
                                identity=identity[0:3, 0:3])
            nmt = npo.tile([128, 1], f32, tag="nmt")
            nc.sync.dma_start(out=nmt[:nvalid],
                              in_=nmask_in[b * NB:b * NB + nvalid, :])
            xo = npo.tile([128, 4], f32, tag="xo")
            nc.vector.scalar_tensor_tensor(
                xo[:, 0:3], agxT_ps[:, 0:3], 1.0 / NORM_FACTOR,
                x_blk[:, b, 0:3], OP.mult, OP.add)
            xom = npo.tile([128, 4], f32, tag="xom")
            nc.vector.tensor_scalar_mul(xom[:, 0:3], xo[:, 0:3], nmt[:])
            nc.sync.dma_start(out=agx_in[b * NB:b * NB + nvalid, :],
                              in_=xom[:nvalid, 0:3])
        assert t == n_tiles

        if collectives:
            with tc.tile_critical():
                cc2 = nc.alloc_semaphore("cc2")
                nc.gpsimd.collective_compute(
                    "AllGather", mybir.AluOpType.bypass,
                    ins=[agx_in[:]], outs=[G2x[:]],
                    replica_groups=[list(range(N_CORES))],
                ).then_inc(cc2, 1)
                nc.gpsimd.wait_ge(cc2, 1)
        else:
            tmpx = npo.tile([128, 4], f32, tag="xo", name="tmpx")
            nc.sync.dma_start(out=tmpx[:, 0:3], in_=agx_in[0:128, :])
            nc.sync.dma_start(out=G2x[0:128, :], in_=tmpx[:, 0:3])

        for i in range(n_node_tiles):
            p = min(128, n_nodes - i * 128)
            tcp = ep.tile([128, 4], f32, tag="xt")
            nc.sync.dma_start(out=tcp[:p, 0:3], in_=G2x[i * 128:i * 128 + p, :])
            nc.sync.dma_start(out=out_x[i * 128:i * 128 + p, :],
                              in_=tcp[:p, 0:3])

    return nc


# ------------------------------------------------------------------ run infra
def make_callable(nc, n_cores=N_CORES):
    import jax
    from jax.sharding import Mesh, PartitionSpec
    from jax.experimental.shard_map import shard_map
    import concourse.mybir as mybir
    from concourse import bass2jax

    bass2jax.install_neuronx_cc_hook()
    partition_name = nc.partition_id_tensor.name if nc.partition_id_tensor else None
    in_names, out_names, out_avals, zero_outs = [], [], [], []
    for alloc in nc.m.functions[0].allocations:
        if not isinstance(alloc, mybir.MemoryLocationSet):
            continue
        name = alloc.memorylocations[0].name
        if alloc.kind == "ExternalInput":
            if name != partition_name:
                in_names.append(name)
        elif alloc.kind == "ExternalOutput":
            out_names.append(name)
            out_avals.append(jax.core.ShapedArray(
                tuple(alloc.tensor_shape), mybir.dt.np(alloc.dtype)))
            zero_outs.append(np.zeros(tuple(alloc.tensor_shape),
                                      mybir.dt.np(alloc.dtype)))
    n_params = len(in_names)
    all_names = in_names + out_names + ([partition_name] if partition_name else [])

    def _body(*args):
        operands = list(args)
        if partition_name is not None:
            operands.append(bass2jax.partition_id_tensor())
        return tuple(bass2jax._bass_exec_p.bind(
            *operands, out_avals=tuple(out_avals), in_names=tuple(all_names),
            out_names=tuple(out_names), lowering_input_output_aliases=(),
            sim_require_finite=False, sim_require_nnan=False, nc=nc))

    mesh = Mesh(np.asarray(jax.devices()[:n_cores]), ("core",))
    n_outs = len(out_names)
    fn = jax.jit(
        shard_map(_body, mesh=mesh,
                  in_specs=(PartitionSpec("core"),) * (n_params + n_outs),
                  out_specs=(PartitionSpec("core"),) * n_outs,
                  check_rep=False),
        keep_unused=True)
    return fn, in_names, out_names, zero_outs, mesh


def prep_in_maps(inputs):
    n_nodes = np.asarray(inputs["h"]).shape[0]
    per_core, meta = host_prep(
        inputs["x"], inputs["edge_index"], inputs["edge_mask"],
        inputs["edge_attr"], n_nodes)
    bcols, brows = make_bias_arrays(inputs)
    h = np.ascontiguousarray(np.asarray(inputs["h"], np.float32))
    x = np.ascontiguousarray(np.asarray(inputs["x"], np.float32))
    nmask = np.asarray(inputs["node_mask"], np.float32).reshape(-1, 1)
    ns, ns_pad = meta["ns"], meta["ns_pad"]
    shared = {
        "h_full": h, "x_full": x,
        "bias_cols": bcols, "bias_rows": brows,
    }
    for k in ("W_lin", "Wa1", "Wa2", "We1", "We2", "Wn1", "Wn2", "Wc1",
              "Wc2", "Wc3"):
        shared[k] = np.ascontiguousarray(np.asarray(inputs[k], np.float32))
    in_maps = []
    for c in range(N_CORES):
        m = dict(shared)
        m.update(per_core[c])
        h_sh = np.zeros((ns_pad, D), np.float32)
        h_sh[:ns] = h[c * ns:(c + 1) * ns]
        m["h_sh"] = h_sh
        m["nmask_sh"] = np.ascontiguousarray(nmask[c * ns:(c + 1) * ns])
        in_maps.append(m)
    return in_maps, meta


def kernel(**inputs):
    in_maps, meta = prep_in_maps(inputs)
    key = (meta["e_pad"], tuple(meta["tiles_per_block"]), meta["n_nodes"])
    if key not in _CACHE:
        nc = build_nc(meta)
        _CACHE[key] = (nc,) + make_callable(nc)
    nc, fn, in_names, out_names, zero_outs, mesh = _CACHE[key]

    import jax
    from jax.sharding import NamedSharding, PartitionSpec
    sh = NamedSharding(mesh, PartitionSpec("core"))
    big_in = [
        jax.device_put(
            np.ascontiguousarray(np.concatenate(
                [np.asarray(in_maps[c][n]) for c in range(N_CORES)], axis=0)),
            sh)
        for n in in_names
    ]
    big_zeros = [
        jax.device_put(np.zeros((N_CORES * z.shape[0], *z.shape[1:]), z.dtype),
                       sh)
        for z in zero_outs
    ]
    outs = fn(*big_in, *big_zeros)
    jax.block_until_ready(outs)
    res = {name: np.asarray(outs[i]).reshape(N_CORES, *zero_outs[i].shape)[0]
           for i, name in enumerate(out_names)}
    _CACHE["last_run"] = (fn, big_in, big_zeros, out_names, zero_outs)
    return res["out_h"], res["out_x"]


def rerun_timed(n_reps=20):
    import jax
    fn, big_in, big_zeros, out_names, zero_outs = _CACHE["last_run"]
    ts = []
    for _ in range(n_reps):
        t0 = time.perf_counter()
        outs = fn(*big_in, *big_zeros)
        jax.block_until_ready(outs)
        ts.append(time.perf_counter() - t0)
    return np.array(ts)


# revision 36
# speedup vs baseline: 1.2638x; 1.2638x over previous
"""Trainium2 Bass kernel for nn_EquivariantBlock (EGNN message-passing block).

Sharding: edges sorted by destination node (row) on the host and sharded by
contiguous node range (N/8 nodes per core) so each core owns every edge of
its node range; per-edge work is edge-parallel, the segment-sum aggregates
are core-local and disjoint, and the two collectives are AllGathers of the
per-core node shards (hh_new, x_out).

Device-side per 512-edge tile:
  - row-side features "gathered" by one-hot matmul from the SBUF-resident
    row block (rows sorted => no DMA),
  - col-side features gathered with [P,1]-offset indirect DMAs from a packed
    DRAM table G1 = [hh | x] (phase 1) / G2 = hh_new (phase 2),
  - edge MLPs run in feature-on-partition (transposed) space on PE,
  - segment sums are one-hot matmuls accumulated in PSUM per 128-node block.
"""

import math
import time
from contextlib import ExitStack

import numpy as np

# ---------------------------------------------------------------- constants
D = 128
N_CORES = 8
NORM_FACTOR = 100.0
NORM_CONST = 1.0
LN_EPS = 1e-5
TILE = 512           # edges per MLP tile
CHUNK = 128          # edges per indirect gather / K<=128 matmul
NCH = TILE // CHUNK  # chunks per tile (4)
NB = 128             # nodes per row block
GROW = D + 8         # G1 row: 128 hh + 3 x + 5 pad (544 B)

_CACHE = {}


# ------------------------------------------------------------------- host prep
def host_prep(x, edge_index, edge_mask, edge_attr, n_nodes):
    """Sort/shard/pad edges; build per-core device input arrays."""
    row = np.asarray(edge_index[0]).astype(np.int64)
    col = np.asarray(edge_index[1]).astype(np.int64)
    emask = np.asarray(edge_mask, np.float32).reshape(-1)
    eattr = np.asarray(edge_attr, np.float32).reshape(-1)

    ns = n_nodes // N_CORES
    n_blocks = math.ceil(ns / NB)
    ns_pad = n_blocks * NB

    x = np.asarray(x, np.float32)
    order = np.argsort(row, kind="stable")
    row_s, col_s = row[order], col[order]
    emask_s, eattr_s = emask[order], eattr[order]

    core_of = row_s // ns
    blk_of = (row_s % ns) // NB
    counts = np.zeros((N_CORES, n_blocks), np.int64)
    np.add.at(counts, (core_of, blk_of), 1)
    tiles_per_block = [
        int(math.ceil(max(1, int(counts[:, b].max())) / TILE))
        for b in range(n_blocks)
    ]
    e_pad = TILE * sum(tiles_per_block)
    n_chunks = e_pad // CHUNK
    n_tiles = e_pad // TILE

    col_idx = np.zeros((N_CORES, n_chunks, CHUNK), np.int32)
    rl = np.zeros((N_CORES, n_chunks, CHUNK), np.float32)
    at = np.zeros((N_CORES, n_chunks, CHUNK), np.float32)
    em = np.zeros((N_CORES, n_chunks, CHUNK), np.float32)
    xr = np.zeros((N_CORES, n_chunks, CHUNK, 3), np.float32)

    starts = np.zeros(N_CORES * n_blocks, np.int64)
    np.cumsum(counts.reshape(-1)[:-1], out=starts[1:])
    starts = starts.reshape(N_CORES, n_blocks)

    for c in range(N_CORES):
        pos = 0
        for b in range(n_blocks):
            s = int(starts[c, b])
            k = int(counts[c, b])
            pe = np.arange(pos, pos + k)
            ch, off = pe // CHUNK, pe % CHUNK
            col_idx[c, ch, off] = col_s[s:s + k]
            rl[c, ch, off] = (row_s[s:s + k] % ns) % NB
            at[c, ch, off] = eattr_s[s:s + k]
            em[c, ch, off] = emask_s[s:s + k]
            xr[c, ch, off, :] = x[row_s[s:s + k]]
            pos += TILE * tiles_per_block[b]
        assert pos == e_pad

    def col_layout(a):
        # [n_chunks, CHUNK] -> [n_tiles, CHUNK, NCH]  (partition-major)
        return np.ascontiguousarray(
            a.reshape(n_tiles, NCH, CHUNK).transpose(0, 2, 1))

    per_core = []
    for c in range(N_CORES):
        x_sh = np.zeros((ns_pad, 3), np.float32)
        x_sh[:ns] = x[c * ns:(c + 1) * ns]
        per_core.append({
            "col_idx": col_layout(col_idx[c]),
            "rl_col": col_layout(rl[c]),
            "em_col": col_layout(em[c]),
            "rlT": np.ascontiguousarray(rl[c].reshape(n_tiles, 1, TILE)),
            "attrT": np.ascontiguousarray(at[c].reshape(n_tiles, 1, TILE)),
            "emT": np.ascontiguousarray(em[c].reshape(n_tiles, 1, TILE)),
            "xr_col": np.ascontiguousarray(
                xr[c].reshape(n_tiles, NCH, CHUNK, 3)
                .transpose(0, 2, 1, 3).reshape(n_tiles, CHUNK, NCH * 3)),
            "x_sh": x_sh,
        })
    meta = dict(n_nodes=n_nodes, ns=ns, ns_pad=ns_pad, n_blocks=n_blocks,
                tiles_per_block=tiles_per_block, e_pad=e_pad,
                n_chunks=n_chunks, n_tiles=n_tiles)
    return per_core, meta


def make_bias_arrays(inp):
    z = np.zeros(D, np.float32)
    cols = np.stack([
        np.asarray(inp.get(k, z), np.float32).reshape(-1) for k in
        ("b_lin", "ba1", "be1", "be2", "bn1", "bn2", "bc1", "bc2")
    ], axis=1)                                    # [128, 8]
    iota_col = np.arange(NB, dtype=np.float32).reshape(NB, 1)
    cols = np.concatenate([cols, iota_col], axis=1)    # [128, 9]
    rows = np.stack([
        np.asarray(inp["ln_g"], np.float32).reshape(-1),
        np.asarray(inp["ln_b"], np.float32).reshape(-1),
        np.full(D, np.float32(np.asarray(inp["ba2"]).reshape(-1)[0])),
        np.arange(D, dtype=np.float32),
    ], axis=0)                                    # [4, 128]
    return cols, rows



# --------------------------------------------------------------- tile patches
# This container's walrus build rejects instructions carrying more than one
# semaphore wait ("Too many sync wait commands").  Redistribute excess waits
# onto single-wait InstNoOp carriers placed just before each instruction on
# the same engine (engine queues are FIFO, so gating is preserved).
_MAX_WAITS = 1
_carrier_n = [0]
_patched = [False]


def _make_carrier(mybir, engine, waits):
    nop = mybir.InstNoOp(name=f"waitcarrier_{_carrier_n[0]}", ins=[], outs=[])
    _carrier_n[0] += 1
    nop.engine = engine
    nop.sync_info = mybir.SyncInfo(on_wait=list(waits), on_update=[])
    return nop


def apply_tile_patch():
    if _patched[0]:
        return
    _patched[0] = True
    import concourse.tile as tile
    import concourse.mybir as mybir

    _orig_lower = tile.TileContext._lower_ordered_insts

    def _patched_lower(self, ordered):
        for bb_name, insts in ordered.items():
            out = []
            for inst in insts:
                si = inst.sync_info
                waits = list(si.on_wait) if si is not None and si.on_wait else []
                if len(waits) > _MAX_WAITS:
                    extra, keep = waits[:-_MAX_WAITS], waits[-_MAX_WAITS:]
                    for k in range(0, len(extra), _MAX_WAITS):
                        out.append(_make_carrier(mybir, inst.engine,
                                                 extra[k:k + _MAX_WAITS]))
                    si.on_wait = keep
                out.append(inst)
            ordered[bb_name] = out
        return _orig_lower(self, ordered)

    def _patched_drain_and_barrier(self, tick_clock, wait_clock):
        from concourse.tile import ScopedClock

        nc = self.nc
        assert self.sems is not None
        allocated = list(self.sems.allocated().values())
        carriers = []
        if allocated:
            for _ in range(48):
                carriers.append(nc.sync.wait_ge(allocated[0], 0))
        drain_inst = nc.sync.drain()
        wait_clock.add_sem_waits(
            drain_inst.ins, ScopedClock({None: tick_clock.global_clock}))
        si = drain_inst.ins.sync_info
        waits = list(si.on_wait) if si and si.on_wait else []
        if len(waits) > 1 and carriers:
            assert len(waits) <= 48, f"need more carriers: {len(waits)}"
            for c, w in zip(carriers, waits[:-1]):
                c.ins.sync_info.on_wait = [w]
            si.on_wait = [waits[-1]]
        nc.all_engine_barrier()
        popped = nc._tile_sem_poison_stack.pop()
        assert popped is self._sem_poison
        nc.clear_and_free_semaphores(allocated)
        nc.all_engine_barrier()

    tile.TileContext._lower_ordered_insts = _patched_lower
    tile.TileContext._drain_and_barrier = _patched_drain_and_barrier


# ------------------------------------------------------------------ bass build
def build_nc(meta, collectives=True):
    import concourse.bass as bass
    import concourse.tile as tile
    from concourse import mybir
    from concourse.masks import make_identity

    apply_tile_patch()

    AF = mybir.ActivationFunctionType
    OP = mybir.AluOpType
    f32 = mybir.dt.float32

    n_nodes = meta["n_nodes"]
    ns = meta["ns"]
    ns_pad = meta["ns_pad"]
    n_blocks = meta["n_blocks"]
    tiles_per_block = meta["tiles_per_block"]
    n_chunks = meta["n_chunks"]
    n_tiles = meta["n_tiles"]
    n_node_tiles = math.ceil(n_nodes / 128)

    nc = bass.Bass("TRN2", target_bir_lowering=False, debug=False,
                   num_devices=N_CORES)

    di = lambda name, shape, dt=f32: nc.dram_tensor(name, shape, dt,
                                                    kind="ExternalInput")
    h_in = di("h_full", [n_nodes, D])
    x_in = di("x_full", [n_nodes, 3])
    h_sh_in = di("h_sh", [ns_pad, D])
    W_lin = di("W_lin", [D, D])
    Wa1 = di("Wa1", [2 * D + 3, D])
    Wa2 = di("Wa2", [D, 1])
    We1 = di("We1", [D + 3, D])
    We2 = di("We2", [D, D])
    Wn1 = di("Wn1", [D, D])
    Wn2 = di("Wn2", [D, D])
    Wc1 = di("Wc1", [2 * D + 3, D])
    Wc2 = di("Wc2", [D, D])
    Wc3 = di("Wc3", [D, 1])
    bcol_in = di("bias_cols", [D, 9])
    brow_in = di("bias_rows", [4, D])
    colx_in = di("col_idx", [n_tiles, CHUNK, NCH], mybir.dt.int32)
    rlc_in = di("rl_col", [n_tiles, CHUNK, NCH])
    emc_in = di("em_col", [n_tiles, CHUNK, NCH])
    rlT_in = di("rlT", [n_tiles, 1, TILE])
    attrT_in = di("attrT", [n_tiles, 1, TILE])
    emT_in = di("emT", [n_tiles, 1, TILE])
    xrc_in = di("xr_col", [n_tiles, CHUNK, NCH * 3])
    xsh_in = di("x_sh", [ns_pad, 3])
    nmask_in = di("nmask_sh", [ns, 1])

    out_h = nc.dram_tensor("out_h", [n_nodes, D], f32, kind="ExternalOutput")
    out_x = nc.dram_tensor("out_x", [n_nodes, 3], f32, kind="ExternalOutput")

    G1 = nc.dram_tensor("G1", [n_nodes, GROW], f32)
    ag_in = nc.dram_tensor("ag_in", [ns, D], f32)
    G2 = nc.dram_tensor("G2", [n_nodes, D], f32, addr_space="Shared")
    agx_in = nc.dram_tensor("agx_in", [ns, 3], f32)
    G2x = nc.dram_tensor("G2x", [n_nodes, 3], f32, addr_space="Shared")
    e3_dram = nc.dram_tensor("e3_dram", [n_tiles, 3, TILE], f32)
    dbg_agg = nc.dram_tensor("dbg_agg", [n_blocks, 128, 128], f32)

    with tile.TileContext(nc) as tc, ExitStack() as ctx:
        singles = ctx.enter_context(tc.tile_pool(name="singles", bufs=1))
        persist = ctx.enter_context(tc.tile_pool(name="persist", bufs=1))
        ep = ctx.enter_context(tc.tile_pool(name="ep", bufs=4))
        epc = ctx.enter_context(tc.tile_pool(name="epc", bufs=2))
        gdst = ctx.enter_context(tc.tile_pool(name="gdst", bufs=6))
        idxt = ctx.enter_context(tc.tile_pool(name="idxt", bufs=6))
        npo = ctx.enter_context(tc.tile_pool(name="npo", bufs=2))
        # PSUM: exactly 8 banks
        ps_gat = ctx.enter_context(tc.tile_pool(name="ps_gat", bufs=1,
                                                space="PSUM"))   # 3 banks
        ps_mlp = ctx.enter_context(tc.tile_pool(name="ps_mlp", bufs=1,
                                                space="PSUM"))   # 2 banks
        ps_sml = ctx.enter_context(tc.tile_pool(name="ps_sml", bufs=1,
                                                space="PSUM"))   # 2 banks
        ps_agg = ctx.enter_context(tc.tile_pool(name="ps_agg", bufs=1,
                                                space="PSUM"))   # 1 bank

        def GAT_HR():
            return ps_gat.tile([128, TILE], f32, tag="gat_hr", name="gat_hr")

        def GAT_HC():
            return ps_gat.tile([128, TILE], f32, tag="gat_hc", name="gat_hc")

        def GAT_MSGT():
            return ps_gat.tile([128, TILE], f32, tag="gat_msgT", name="gat_msgT")

        def MLP_A():
            return ps_mlp.tile([128, TILE], f32, tag="mlp_A", name="mlp_A",
                               bufs=2)

        def MLP_B():
            return ps_mlp.tile([128, TILE], f32, tag="mlp_B", name="mlp_B")

        def SML_ROW():
            return ps_sml.tile([1, TILE], f32, tag="sml_row", name="sml_row")

        # ---------------- weights / constants ----------------
        _wn = [0]

        def wtile(ap, shape):
            nm = f"wt{_wn[0]}"
            _wn[0] += 1
            t = singles.tile(shape, f32, name=nm, tag=nm)
            nc.sync.dma_start(out=t[:], in_=ap[:])
            return t

        w_lin = wtile(W_lin, [D, D])
        wa1a = wtile(Wa1[0:D, :], [D, D])
        wa1b = wtile(Wa1[D:2 * D, :], [D, D])
        wa1c = wtile(Wa1[2 * D:2 * D + 3, :], [3, D])
        wa2 = wtile(Wa2, [D, 1])
        we1a = wtile(We1[0:D, :], [D, D])
        we1b = wtile(We1[D:D + 3, :], [3, D])
        we2 = wtile(We2, [D, D])
        wn1 = wtile(Wn1, [D, D])
        wn2 = wtile(Wn2, [D, D])
        wc1a = wtile(Wc1[0:D, :], [D, D])
        wc1b = wtile(Wc1[D:2 * D, :], [D, D])
        wc1c = wtile(Wc1[2 * D:2 * D + 3, :], [3, D])
        wc2 = wtile(Wc2, [D, D])
        wc3 = wtile(Wc3, [D, 1])
        bc = wtile(bcol_in, [D, 9])
        b_lin, ba1, be1, be2 = bc[:, 0:1], bc[:, 1:2], bc[:, 2:3], bc[:, 3:4]
        bn1, bn2, bc1, bc2 = bc[:, 4:5], bc[:, 5:6], bc[:, 6:7], bc[:, 7:8]
        iota_col = bc[:, 8:9]
        ln_g = wtile(brow_in[0:1, :], [1, D])
        ln_b = wtile(brow_in[1:2, :], [1, D])
        ba2_row = wtile(brow_in[2:3, :], [1, D])
        iota_row = wtile(brow_in[3:4, :], [1, D])
        ba2_s = ba2_row[0:1, 0:1]

        identity = singles.tile([128, 128], f32)
        make_identity(nc, identity[:])
        ones_col = singles.tile([128, 1], f32)
        nc.vector.memset(ones_col[:], 1.0)
        ones_row = singles.tile([1, 128], f32)
        nc.vector.memset(ones_row[:], 1.0)
        eps_col = singles.tile([128, 1], f32)
        nc.vector.memset(eps_col[:], LN_EPS)
        eps8_col = singles.tile([128, 1], f32)
        nc.vector.memset(eps8_col[:], 1e-8)

        # broadcast constants: iota / ln_g / ln_b replicated to all partitions
        iota_bc = singles.tile([128, 128], f32)
        lng_bc = singles.tile([128, 128], f32)
        lnb_bc = singles.tile([128, 128], f32)
        bc_ps = GAT_HR()
        nc.tensor.matmul(bc_ps[:, 0:128], ones_row[:], iota_row[:],
                         start=True, stop=True)
        nc.vector.tensor_copy(iota_bc[:], bc_ps[:, 0:128])
        bc_ps2 = GAT_HC()
        nc.tensor.matmul(bc_ps2[:, 0:128], ones_row[:], ln_g[:],
                         start=True, stop=True)
        nc.vector.tensor_copy(lng_bc[:], bc_ps2[:, 0:128])
        bc_ps3 = GAT_MSGT()
        nc.tensor.matmul(bc_ps3[:, 0:128], ones_row[:], ln_b[:],
                         start=True, stop=True)
        nc.vector.tensor_copy(lnb_bc[:], bc_ps3[:, 0:128])

        # persisted per-edge / per-node data
        cd_sb = persist.tile([128, n_chunks, 4], f32)            # coord_diff
        hhn_sb = persist.tile([128, n_blocks, 128], f32)         # hh_new rows
        hh_own = persist.tile([128, n_blocks, 128], f32)         # hh own rows
        x_blk = persist.tile([128, n_blocks, 4], f32)            # x own rows
        agg_sb = persist.tile([128, n_blocks, 128], f32)         # aggT

        # ============ prologue A: hh = h @ W_lin + b_lin -> G1 ============
        for i in range(n_node_tiles):
            p = min(128, n_nodes - i * 128)
            ht = ep.tile([128, D], f32, tag="ht")
            nc.sync.dma_start(out=ht[:p], in_=h_in[i * 128:i * 128 + p, :])
            hT_ps = GAT_HR()
            nc.tensor.transpose(out=hT_ps[:128, :p], in_=ht[:p, :],
                                identity=identity[:p, :p])
            hT = epc.tile([128, 128], f32, tag="w_hT")
            nc.scalar.activation(hT[:, :p], hT_ps[:, :p], AF.Copy)
            hhT_ps = MLP_A()
            nc.tensor.matmul(hhT_ps[:, :p], w_lin[:], hT[:, :p],
                             start=True, stop=True)
            hhT = epc.tile([128, 128], f32, tag="w_hhT")
            nc.scalar.activation(hhT[:, :p], hhT_ps[:, :p], AF.Identity,
                                 bias=b_lin)
            hh_ps = GAT_HC()
            nc.tensor.transpose(out=hh_ps[:p, 0:128], in_=hhT[:, :p],
                                identity=identity[:])
            gt = epc.tile([128, GROW], f32, tag="w_gt")
            nc.vector.tensor_copy(gt[:p, 0:D], hh_ps[:p, 0:128])
            xt = ep.tile([128, 4], f32, tag="xt")
            nc.sync.dma_start(out=xt[:p, 0:3], in_=x_in[i * 128:i * 128 + p, :])
            nc.vector.tensor_copy(gt[:p, D:D + 3], xt[:p, 0:3])
            nc.vector.memset(gt[:p, D + 3:GROW], 0.0)
            nc.sync.dma_start(out=G1[i * 128:i * 128 + p, :], in_=gt[:p, :])

        # ============ prologue B: hh/x for own row blocks (SBUF) ============
        for b in range(n_blocks):
            hsb = ep.tile([128, D], f32, tag="ht")
            nc.sync.dma_start(out=hsb[:], in_=h_sh_in[b * NB:(b + 1) * NB, :])
            hT_ps = GAT_HR()
            nc.tensor.transpose(out=hT_ps[:, 0:128], in_=hsb[:],
                                identity=identity[:])
            hT = epc.tile([128, 128], f32, tag="w_hT")
            nc.scalar.activation(hT[:], hT_ps[:, 0:128], AF.Copy)
            hhT_ps = MLP_A()
            nc.tensor.matmul(hhT_ps[:, 0:128], w_lin[:], hT[:],
                             start=True, stop=True)
            hhT = epc.tile([128, 128], f32, tag="w_hhT")
            nc.scalar.activation(hhT[:], hhT_ps[:, 0:128], AF.Identity,
                                 bias=b_lin)
            hh_ps = GAT_HC()
            nc.tensor.transpose(out=hh_ps[:, 0:128], in_=hhT[:],
                                identity=identity[:])
            nc.vector.tensor_copy(hh_own[:, b, :], hh_ps[:, 0:128])
            xbt = ep.tile([128, 4], f32, tag="xt")
            nc.sync.dma_start(out=xbt[:, 0:3],
                              in_=xsh_in[b * NB:(b + 1) * NB, :])
            nc.vector.tensor_copy(x_blk[:, b, 0:3], xbt[:, 0:3])

        # ================= phase 1: GCLayer edge pass =================
        def edge_tile_phase1(t, b, start, stop, agg_ps):
            c0 = t * NCH
            rlT_t = ep.tile([1, TILE], f32, tag="rlT")
            nc.sync.dma_start(out=rlT_t[:], in_=rlT_in[t])
            rlc_t = ep.tile([128, NCH], f32, tag="rlc")
            nc.sync.dma_start(out=rlc_t[:], in_=rlc_in[t])
            emc_t = ep.tile([128, NCH], f32, tag="emc")
            nc.sync.dma_start(out=emc_t[:], in_=emc_in[t])
            idx_t = idxt.tile([128, NCH], mybir.dt.int32, tag="idx")
            nc.sync.dma_start(out=idx_t[:], in_=colx_in[t])

            graw = gdst.tile([128, NCH, GROW], f32, tag="graw")
            for c in range(NCH):
                nc.gpsimd.indirect_dma_start(
                    out=graw[:, c, :], out_offset=None, in_=G1[:],
                    in_offset=bass.IndirectOffsetOnAxis(
                        ap=idx_t[:, c:c + 1], axis=0),
                )

            rlbc_sb = epc.tile([128, TILE], f32, tag="rlbc", name="rlbc_sb")
            rl_bcast_ap = bass.AP(
                tensor=rlT_in[t].tensor, offset=rlT_in[t].offset,
                ap=[[0, 128], [1, TILE]])
            nc.sync.dma_start(out=rlbc_sb[:], in_=rl_bcast_ap)
            onehotT = epc.tile([128, TILE], f32, tag="onehotT")
            nc.vector.tensor_scalar(
                onehotT[:], rlbc_sb[:], iota_col, None, OP.is_equal)

            hrT_ps = GAT_HR()
            nc.tensor.matmul(hrT_ps[:], hh_own[:, b, :], onehotT[:],
                             start=True, stop=True)
            hrT = epc.tile([128, TILE], f32, tag="hrT")
            nc.scalar.activation(hrT[:], hrT_ps[:], AF.Copy)

            hcT_ps = GAT_HC()
            for c in range(NCH):
                nc.tensor.transpose(out=hcT_ps[:, c * 128:(c + 1) * 128],
                                    in_=graw[:, c, 0:D], identity=identity[:])
            hcT = epc.tile([128, TILE], f32, tag="hcT")
            nc.vector.tensor_copy(hcT[:], hcT_ps[:])

            dT = epc.tile([128, TILE], f32, tag="dT")
            nc.vector.tensor_sub(dT[:], hcT[:], hrT[:])
            sqT = epc.tile([128, TILE], f32, tag="sqT")
            nc.scalar.activation(sqT[:], dT[:], AF.Square)

            geo2_ps = SML_ROW()
            nc.tensor.matmul(geo2_ps[:], ones_col[:], sqT[:],
                             start=True, stop=True)
            e3 = epc.tile([4, TILE], f32, tag="e3cur", name="e3cur")
            georow = ep.tile([1, TILE], f32, tag="georow")
            nc.scalar.activation(georow[:], geo2_ps[:], AF.Sqrt,
                                 bias=eps8_col[0:1, :])
            nc.sync.dma_start(out=e3[2:3, :], in_=georow[:])
            nc.sync.dma_start(out=e3[1:2, :], in_=attrT_in[t])

            # coord path in [e, 3] layout, per chunk (xr host-gathered)
            xrc_t = ep.tile([128, NCH * 3], f32, tag="xrc")
            nc.sync.dma_start(out=xrc_t[:], in_=xrc_in[t])
            dist_ps = SML_ROW()
            diff = ep.tile([128, NCH, 3], f32, tag="diff")
            nc.vector.tensor_sub(
                diff[:], xrc_t[:].rearrange("p (c k) -> p c k", k=3),
                graw[:, :, D:D + 3])
            rad = ep.tile([128, NCH], f32, tag="rad")
            sqd = ep.tile([128, NCH, 3], f32, tag="sqd")
            for c in range(NCH):
                nc.scalar.activation(sqd[:, c, :], diff[:, c, :], AF.Square,
                                     accum_out=rad[:, c:c + 1])
            dist_c = ep.tile([128, NCH], f32, tag="dist")
            nc.scalar.activation(dist_c[:], rad[:], AF.Sqrt, bias=eps8_col[:])
            den = ep.tile([128, NCH], f32, tag="den")
            nc.vector.tensor_scalar_add(den[:], dist_c[:], NORM_CONST)
            rec = ep.tile([128, NCH], f32, tag="rec")
            nc.vector.reciprocal(rec[:], den[:])
            for c in range(NCH):
                nc.vector.tensor_scalar_mul(cd_sb[:, c0 + c, 0:3],
                                            diff[:, c, :], rec[:, c:c + 1])
                nc.tensor.transpose(out=dist_ps[:, c * 128:(c + 1) * 128],
                                    in_=dist_c[:, c:c + 1],
                                    identity=identity[:])
            nc.vector.tensor_copy(e3[0:1, :], dist_ps[:])
            nc.sync.dma_start(out=e3_dram[t], in_=e3[0:3, :])

            att1_ps = MLP_A()
            nc.tensor.matmul(att1_ps[:], wa1a[:], hrT[:], start=True, stop=False)
            nc.tensor.matmul(att1_ps[:], wa1b[:], hcT[:], start=False,
                             stop=False)
            nc.tensor.matmul(att1_ps[:], wa1c[:], e3[0:3, :], start=False,
                             stop=True)
            satt1 = epc.tile([128, TILE], f32, tag="satt1")
            nc.scalar.activation(satt1[:], att1_ps[:], AF.Silu, bias=ba1)
            att2_ps = SML_ROW()
            nc.tensor.matmul(att2_ps[:], wa2[:], satt1[:],
                             start=True, stop=True)
            attT = ep.tile([1, TILE], f32, tag="attT")
            nc.scalar.activation(attT[:], att2_ps[:], AF.Sigmoid,
                                 bias=ba2_s)

            msg1_ps = MLP_B()
            nc.tensor.matmul(msg1_ps[:], we1a[:], dT[:], start=True, stop=False)
            nc.tensor.matmul(msg1_ps[:], we1b[:], e3[0:3, :], start=False,
                             stop=True)
            smsg1 = epc.tile([128, TILE], f32, tag="smsg1")
            nc.scalar.activation(smsg1[:], msg1_ps[:], AF.Silu, bias=be1)
            msg2_ps = MLP_B()
            nc.tensor.matmul(msg2_ps[:], we2[:], smsg1[:], start=True,
                             stop=True)
            msgb = epc.tile([128, TILE], f32, tag="msgb")
            nc.scalar.activation(msgb[:], msg2_ps[:], AF.Identity, bias=be2)
            # fold att * edge_mask into msg in transposed space
            emT_t = ep.tile([1, TILE], f32, tag="emT")
            nc.sync.dma_start(out=emT_t[:], in_=emT_in[t])
            attm = ep.tile([1, TILE], f32, tag="attm")
            nc.vector.tensor_tensor(attm[:], attT[:], emT_t[:], OP.mult)
            attbc_ps = GAT_MSGT()
            nc.tensor.matmul(attbc_ps[:], ones_row[:], attm[:],
                             start=True, stop=True)
            msgs = epc.tile([128, TILE], f32, tag="msgs")
            nc.vector.tensor_tensor(msgs[:], msgb[:], attbc_ps[:], OP.mult)

            msgT_ps = GAT_MSGT()
            for c in range(NCH):
                nc.tensor.transpose(out=msgT_ps[:, c * 128:(c + 1) * 128],
                                    in_=msgs[:, c * 128:(c + 1) * 128],
                                    identity=identity[:])
            msg_sb = epc.tile([128, NCH, 128], f32, tag="msg_sb")
            nc.vector.tensor_copy(msg_sb[:], msgT_ps[:])
            for c in range(NCH):
                onehot = epc.tile([128, NCH, 128], f32, tag="onehot")
                nc.vector.tensor_scalar(
                    onehot[:, c, :], iota_bc[:],
                    rlc_t[:, c:c + 1], None, OP.is_equal)
                nc.tensor.matmul(agg_ps[:], msg_sb[:, c, :], onehot[:, c, :],
                                 start=(start and c == 0),
                                 stop=(stop and c == NCH - 1))

        t = 0
        for b in range(n_blocks):
            agg_ps = ps_agg.tile([128, 128], f32, tag="agg")
            for k in range(tiles_per_block[b]):
                edge_tile_phase1(t, b, start=(k == 0),
                                 stop=(k == tiles_per_block[b] - 1),
                                 agg_ps=agg_ps)
                t += 1
            nc.vector.tensor_copy(agg_sb[:, b, :], agg_ps[:])
            nc.sync.dma_start(out=dbg_agg[b], in_=agg_sb[:, b, :])
        assert t == n_tiles

        # ================= node step: MLP + LN + silu, AllGather ==========
        for b in range(n_blocks):
            nvalid = min(NB, ns - b * NB)
            z1_ps = MLP_A()
            nc.tensor.matmul(z1_ps[:, 0:128], wn1[:], agg_sb[:, b, :],
                             start=True, stop=True)
            sz1 = npo.tile([128, 128], f32, tag="sz1")
            nc.scalar.activation(sz1[:], z1_ps[:, 0:128], AF.Silu, bias=bn1)
            z2_ps = MLP_B()
            nc.tensor.matmul(z2_ps[:, 0:128], wn2[:], sz1[:], start=True,
                             stop=True)
            z2T = npo.tile([128, 128], f32, tag="z2T")
            nc.scalar.activation(z2T[:], z2_ps[:, 0:128], AF.Identity,
                                 bias=bn2)
            z2n_ps = GAT_HR()
            nc.tensor.transpose(out=z2n_ps[:, 0:128], in_=z2T[:],
                                identity=identity[:])
            s = npo.tile([128, 128], f32, tag="s")
            nc.vector.tensor_add(s[:], z2n_ps[:, 0:128], hh_own[:, b, :])
            stats = npo.tile([128, 6], f32, tag="stats")
            nc.vector.bn_stats(out=stats[:], in_=s[:])
            mv = npo.tile([128, 2], f32, tag="mv")
            nc.vector.bn_aggr(out=mv[:], in_=stats[:])
            sd = npo.tile([128, 1], f32, tag="sd")
            nc.scalar.activation(sd[:], mv[:, 1:2], AF.Sqrt, bias=eps_col)
            rstd = npo.tile([128, 1], f32, tag="rstd")
            nc.vector.reciprocal(rstd[:], sd[:])
            y = npo.tile([128, 128], f32, tag="y")
            nc.vector.tensor_scalar(y[:], s[:], mv[:, 0:1], rstd[:],
                                    OP.subtract, OP.mult)
            yg = npo.tile([128, 128], f32, tag="yg")
            nc.vector.tensor_tensor(yg[:], y[:], lng_bc[:], OP.mult)
            yb = npo.tile([128, 128], f32, tag="yb")
            nc.vector.tensor_tensor(yb[:], yg[:], lnb_bc[:], OP.add)
            nc.scalar.activation(hhn_sb[:, b, :], yb[:], AF.Silu)
            nc.sync.dma_start(out=ag_in[b * NB:b * NB + nvalid, :],
                              in_=hhn_sb[:nvalid, b, :])

        if collectives:
            with tc.tile_critical():
                cc1 = nc.alloc_semaphore("cc1")
                nc.gpsimd.collective_compute(
                    "AllGather", mybir.AluOpType.bypass,
                    ins=[ag_in[:]], outs=[G2[:]],
                    replica_groups=[list(range(N_CORES))],
                ).then_inc(cc1, 1)
                nc.gpsimd.wait_ge(cc1, 1)
        else:
            for bb in range(n_blocks):
                nv = min(NB, ns - bb * NB)
                tmpg = npo.tile([128, D], f32, tag="sz1", name="tmpg")
                nc.sync.dma_start(out=tmpg[:nv], in_=ag_in[bb * NB:bb * NB + nv, :])
                nc.sync.dma_start(out=G2[bb * NB:bb * NB + nv, :],
                                  in_=tmpg[:nv])

        for i in range(n_node_tiles):
            p = min(128, n_nodes - i * 128)
            tcp = ep.tile([128, D], f32, tag="ht")
            nc.sync.dma_start(out=tcp[:p], in_=G2[i * 128:i * 128 + p, :])
            nc.sync.dma_start(out=out_h[i * 128:i * 128 + p, :], in_=tcp[:p])

        # ================= phase 2: coord MLP edge pass ===================
        def edge_tile_phase2(t, b, start, stop, agx_ps):
            c0 = t * NCH
            rlT_t = ep.tile([1, TILE], f32, tag="rlT")
            nc.sync.dma_start(out=rlT_t[:], in_=rlT_in[t])
            rlc_t = ep.tile([128, NCH], f32, tag="rlc")
            nc.sync.dma_start(out=rlc_t[:], in_=rlc_in[t])
            emc_t = ep.tile([128, NCH], f32, tag="emc")
            nc.sync.dma_start(out=emc_t[:], in_=emc_in[t])
            idx_t = idxt.tile([128, NCH], mybir.dt.int32, tag="idx")
            nc.sync.dma_start(out=idx_t[:], in_=colx_in[t])

            graw = gdst.tile([128, NCH, D], f32, tag="graw2")
            for c in range(NCH):
                nc.gpsimd.indirect_dma_start(
                    out=graw[:, c, :], out_offset=None, in_=G2[:],
                    in_offset=bass.IndirectOffsetOnAxis(
                        ap=idx_t[:, c:c + 1], axis=0),
                )

            rlbc_sb = epc.tile([128, TILE], f32, tag="rlbc", name="rlbc_sb")
            rl_bcast_ap = bass.AP(
                tensor=rlT_in[t].tensor, offset=rlT_in[t].offset,
                ap=[[0, 128], [1, TILE]])
            nc.sync.dma_start(out=rlbc_sb[:], in_=rl_bcast_ap)
            onehotT = epc.tile([128, TILE], f32, tag="onehotT")
            nc.vector.tensor_scalar(
                onehotT[:], rlbc_sb[:], iota_col, None, OP.is_equal)

            grT_ps = GAT_HR()
            nc.tensor.matmul(grT_ps[:], hhn_sb[:, b, :], onehotT[:],
                             start=True, stop=True)
            grT = epc.tile([128, TILE], f32, tag="hrT")
            nc.scalar.activation(grT[:], grT_ps[:], AF.Copy)

            gcT_ps = GAT_HC()
            for c in range(NCH):
                nc.tensor.transpose(out=gcT_ps[:, c * 128:(c + 1) * 128],
                                    in_=graw[:, c, :], identity=identity[:])
            gcT = epc.tile([128, TILE], f32, tag="hcT")
            nc.vector.tensor_copy(gcT[:], gcT_ps[:])

            e3 = epc.tile([4, TILE], f32, tag="e3cur", name="e3cur")
            nc.sync.dma_start(out=e3[0:3, :], in_=e3_dram[t])
            m1_ps = MLP_A()
            nc.tensor.matmul(m1_ps[:], wc1a[:], grT[:], start=True, stop=False)
            nc.tensor.matmul(m1_ps[:], wc1b[:], gcT[:], start=False,
                             stop=False)
            nc.tensor.matmul(m1_ps[:], wc1c[:], e3[0:3, :], start=False,
                             stop=True)
            sm1 = epc.tile([128, TILE], f32, tag="satt1")
            nc.scalar.activation(sm1[:], m1_ps[:], AF.Silu, bias=bc1)
            m2_ps = MLP_B()
            nc.tensor.matmul(m2_ps[:], wc2[:], sm1[:], start=True, stop=True)
            sm2 = epc.tile([128, TILE], f32, tag="smsg1")
            nc.scalar.activation(sm2[:], m2_ps[:], AF.Silu, bias=bc2)
            mT_ps = SML_ROW()
            nc.tensor.matmul(mT_ps[:], wc3[:], sm2[:], start=True, stop=True)
            mrow = ep.tile([1, TILE], f32, tag="attT")
            nc.vector.tensor_copy(mrow[:], mT_ps[:])

            mc_ps = ps_gat.tile([128, TILE], f32, tag="gat_msgT",
                                name="mc_ps")
            for c in range(NCH):
                nc.tensor.transpose(out=mc_ps[:, c:c + 1],
                                    in_=mrow[:, c * 128:(c + 1) * 128],
                                    identity=identity[0:1, 0:1])
            fac = ep.tile([128, NCH], f32, tag="fac")
            nc.vector.tensor_tensor(fac[:], mc_ps[:, 0:NCH], emc_t[:],
                                    OP.mult)
            for c in range(NCH):
                trans = epc.tile([128, NCH, 4], f32, tag="trans")
                nc.vector.tensor_scalar_mul(trans[:, c, 0:3],
                                            cd_sb[:, c0 + c, 0:3],
                                            fac[:, c:c + 1])
                onehot = epc.tile([128, NCH, 128], f32, tag="onehot")
                nc.vector.tensor_scalar(
                    onehot[:, c, :], iota_bc[:],
                    rlc_t[:, c:c + 1], None, OP.is_equal)
                nc.tensor.matmul(agx_ps[0:3, :], trans[:, c, 0:3],
                                 onehot[:, c, :],
                                 start=(start and c == 0),
                                 stop=(stop and c == NCH - 1))

        t = 0
        for b in range(n_blocks):
            nvalid = min(NB, ns - b * NB)
            agx_ps = ps_agg.tile([128, 128], f32, tag="agg")
            for k in range(tiles_per_block[b]):
                edge_tile_phase2(t, b, start=(k == 0),
                                 stop=(k == tiles_per_block[b] - 1),
                                 agx_ps=agx_ps)
                t += 1
            agx_sb = npo.tile([4, 128], f32, tag="agx_sb")
            nc.vector.tensor_copy(agx_sb[0:3, :], agx_ps[0:3, :])
            agxT_ps = GAT_HR()
            nc.tensor.transpose(out=agxT_ps[:, 0:3], in_=agx_sb[0:3, :],
                                identity=identity[0:3, 0:3])
            nmt = npo.tile([128, 1], f32, tag="nmt")
            nc.sync.dma_start(out=nmt[:nvalid],
                              in_=nmask_in[b * NB:b * NB + nvalid, :])
            xo = npo.tile([128, 4], f32, tag="xo")
            nc.vector.scalar_tensor_tensor(
                xo[:, 0:3], agxT_ps[:, 0:3], 1.0 / NORM_FACTOR,
                x_blk[:, b, 0:3], OP.mult, OP.add)
            xom = npo.tile([128, 4], f32, tag="xom")
            nc.vector.tensor_scalar_mul(xom[:, 0:3], xo[:, 0:3], nmt[:])
            nc.sync.dma_start(out=agx_in[b * NB:b * NB + nvalid, :],
                              in_=xom[:nvalid, 0:3])
        assert t == n_tiles

        if collectives:
            with tc.tile_critical():
                cc2 = nc.alloc_semaphore("cc2")
                nc.gpsimd.collective_compute(
                    "AllGather", mybir.AluOpType.bypass,
                    ins=[agx_in[:]], outs=[G2x[:]],
                    replica_groups=[list(range(N_CORES))],
                ).then_inc(cc2, 1)
                nc.gpsimd.wait_ge(cc2, 1)
        else:
            tmpx = npo.tile([128, 4], f32, tag="xo", name="tmpx")
            nc.sync.dma_start(out=tmpx[:, 0:3], in_=agx_in[0:128, :])
            nc.sync.dma_start(out=G2x[0:128, :], in_=tmpx[:, 0:3])

        for i in range(n_node_tiles):
            p = min(128, n_nodes - i * 128)
            tcp = ep.tile([128, 4], f32, tag="xt")
            nc.sync.dma_start(out=tcp[:p, 0:3], in_=G2x[i * 128:i * 128 + p, :])
            nc.sync.dma_start(out=out_x[i * 128:i * 128 + p, :],
                              in_=tcp[:p, 0:3])

    return nc


# ------------------------------------------------------------------ run infra
def make_callable(nc, n_cores=N_CORES):
    import jax
    from jax.sharding import Mesh, PartitionSpec
    from jax.experimental.shard_map import shard_map
    import concourse.mybir as mybir
    from concourse import bass2jax

    bass2jax.install_neuronx_cc_hook()
    partition_name = nc.partition_id_tensor.name if nc.partition_id_tensor else None
    in_names, out_names, out_avals, zero_outs = [], [], [], []
    for alloc in nc.m.functions[0].allocations:
        if not isinstance(alloc, mybir.MemoryLocationSet):
            continue
        name = alloc.memorylocations[0].name
        if alloc.kind == "ExternalInput":
            if name != partition_name:
                in_names.append(name)
        elif alloc.kind == "ExternalOutput":
            out_names.append(name)
            out_avals.append(jax.core.ShapedArray(
                tuple(alloc.tensor_shape), mybir.dt.np(alloc.dtype)))
            zero_outs.append(np.zeros(tuple(alloc.tensor_shape),
                                      mybir.dt.np(alloc.dtype)))
    n_params = len(in_names)
    all_names = in_names + out_names + ([partition_name] if partition_name else [])

    def _body(*args):
        operands = list(args)
        if partition_name is not None:
            operands.append(bass2jax.partition_id_tensor())
        return tuple(bass2jax._bass_exec_p.bind(
            *operands, out_avals=tuple(out_avals), in_names=tuple(all_names),
            out_names=tuple(out_names), lowering_input_output_aliases=(),
            sim_require_finite=False, sim_require_nnan=False, nc=nc))

    mesh = Mesh(np.asarray(jax.devices()[:n_cores]), ("core",))
    n_outs = len(out_names)
    fn = jax.jit(
        shard_map(_body, mesh=mesh,
                  in_specs=(PartitionSpec("core"),) * (n_params + n_outs),
                  out_specs=(PartitionSpec("core"),) * n_outs,
                  check_rep=False),
        keep_unused=True)
    return fn, in_names, out_names, zero_outs, mesh


def prep_in_maps(inputs):
    n_nodes = np.asarray(inputs["h"]).shape[0]
    per_core, meta = host_prep(
        inputs["x"], inputs["edge_index"], inputs["edge_mask"],
        inputs["edge_attr"], n_nodes)
    bcols, brows = make_bias_arrays(inputs)
    h = np.ascontiguousarray(np.asarray(inputs["h"], np.float32))
    x = np.ascontiguousarray(np.asarray(inputs["x"], np.float32))
    nmask = np.asarray(inputs["node_mask"], np.float32).reshape(-1, 1)
    ns, ns_pad = meta["ns"], meta["ns_pad"]
    shared = {
        "h_full": h, "x_full": x,
        "bias_cols": bcols, "bias_rows": brows,
    }
    for k in ("W_lin", "Wa1", "Wa2", "We1", "We2", "Wn1", "Wn2", "Wc1",
              "Wc2", "Wc3"):
        shared[k] = np.ascontiguousarray(np.asarray(inputs[k], np.float32))
    in_maps = []
    for c in range(N_CORES):
        m = dict(shared)
        m.update(per_core[c])
        h_sh = np.zeros((ns_pad, D), np.float32)
        h_sh[:ns] = h[c * ns:(c + 1) * ns]
        m["h_sh"] = h_sh
        m["nmask_sh"] = np.ascontiguousarray(nmask[c * ns:(c + 1) * ns])
        in_maps.append(m)
    return in_maps, meta


def kernel(**inputs):
    in_maps, meta = prep_in_maps(inputs)
    key = (meta["e_pad"], tuple(meta["tiles_per_block"]), meta["n_nodes"])
    if key not in _CACHE:
        nc = build_nc(meta)
        _CACHE[key] = (nc,) + make_callable(nc)
    nc, fn, in_names, out_names, zero_outs, mesh = _CACHE[key]

    import jax
    from jax.sharding import NamedSharding, PartitionSpec
    sh = NamedSharding(mesh, PartitionSpec("core"))
    big_in = [
        jax.device_put(
            np.ascontiguousarray(np.concatenate(
                [np.asarray(in_maps[c][n]) for c in range(N_CORES)], axis=0)),
            sh)
        for n in in_names
    ]
    big_zeros = [
        jax.device_put(np.zeros((N_CORES * z.shape[0], *z.shape[1:]), z.dtype),
                       sh)
        for z in zero_outs
    ]
    outs = fn(*big_in, *big_zeros)
    jax.block_until_ready(outs)
    res = {name: np.asarray(outs[i]).reshape(N_CORES, *zero_outs[i].shape)[0]
           for i, name in enumerate(out_names)}
    _CACHE["last_run"] = (fn, big_in, big_zeros, out_names, zero_outs)
    return res["out_h"], res["out_x"]


def rerun_timed(n_reps=20):
    import jax
    fn, big_in, big_zeros, out_names, zero_outs = _CACHE["last_run"]
    ts = []
    for _ in range(n_reps):
        t0 = time.perf_counter()
        outs = fn(*big_in, *big_zeros)
        jax.block_until_ready(outs)
        ts.append(time.perf_counter() - t0)
    return np.array(ts)


# revision 37
# speedup vs baseline: 218.7470x; 173.0865x over previous
"""Trainium2 Bass kernel for nn_EquivariantBlock (EGNN message-passing block).

Sharding: edges sorted by destination node (row) on the host and sharded by
contiguous node range (N/8 nodes per core) so each core owns every edge of
its node range; per-edge work is edge-parallel, the segment-sum aggregates
are core-local and disjoint, and the two collectives are AllGathers of the
per-core node shards (hh_new, x_out).

Device-side per 512-edge tile:
  - row-side features "gathered" by one-hot matmul from the SBUF-resident
    row block (rows sorted => no DMA),
  - col-side features gathered with [P,1]-offset indirect DMAs from a packed
    DRAM table G1 = [hh | x] (phase 1) / G2 = hh_new (phase 2),
  - edge MLPs run in feature-on-partition (transposed) space on PE,
  - segment sums are one-hot matmuls accumulated in PSUM per 128-node block.
"""

import math
import time
from contextlib import ExitStack

import numpy as np

# ---------------------------------------------------------------- constants
D = 128
N_CORES = 8
NORM_FACTOR = 100.0
NORM_CONST = 1.0
LN_EPS = 1e-5
TILE = 512           # edges per MLP tile
CHUNK = 128          # edges per indirect gather / K<=128 matmul
NCH = TILE // CHUNK  # chunks per tile (4)
NB = 128             # nodes per row block
GROW = D + 8         # G1 row: 128 hh + 3 x + 5 pad (544 B)

_CACHE = {}


# ------------------------------------------------------------------- host prep
def host_prep(x, edge_index, edge_mask, edge_attr, n_nodes):
    """Sort/shard/pad edges; build per-core device input arrays."""
    row = np.asarray(edge_index[0]).astype(np.int64)
    col = np.asarray(edge_index[1]).astype(np.int64)
    emask = np.asarray(edge_mask, np.float32).reshape(-1)
    eattr = np.asarray(edge_attr, np.float32).reshape(-1)

    ns = n_nodes // N_CORES
    n_blocks = math.ceil(ns / NB)
    ns_pad = n_blocks * NB

    x = np.asarray(x, np.float32)
    order = np.argsort(row, kind="stable")
    row_s, col_s = row[order], col[order]
    emask_s, eattr_s = emask[order], eattr[order]

    core_of = row_s // ns
    blk_of = (row_s % ns) // NB
    counts = np.zeros((N_CORES, n_blocks), np.int64)
    np.add.at(counts, (core_of, blk_of), 1)
    tiles_per_block = [
        int(math.ceil(max(1, int(counts[:, b].max())) / TILE))
        for b in range(n_blocks)
    ]
    e_pad = TILE * sum(tiles_per_block)
    n_chunks = e_pad // CHUNK
    n_tiles = e_pad // TILE

    col_idx = np.zeros((N_CORES, n_chunks, CHUNK), np.int32)
    rl = np.zeros((N_CORES, n_chunks, CHUNK), np.float32)
    at = np.zeros((N_CORES, n_chunks, CHUNK), np.float32)
    em = np.zeros((N_CORES, n_chunks, CHUNK), np.float32)
    xr = np.zeros((N_CORES, n_chunks, CHUNK, 3), np.float32)

    starts = np.zeros(N_CORES * n_blocks, np.int64)
    np.cumsum(counts.reshape(-1)[:-1], out=starts[1:])
    starts = starts.reshape(N_CORES, n_blocks)

    for c in range(N_CORES):
        pos = 0
        for b in range(n_blocks):
            s = int(starts[c, b])
            k = int(counts[c, b])
            pe = np.arange(pos, pos + k)
            ch, off = pe // CHUNK, pe % CHUNK
            col_idx[c, ch, off] = col_s[s:s + k]
            rl[c, ch, off] = (row_s[s:s + k] % ns) % NB
            at[c, ch, off] = eattr_s[s:s + k]
            em[c, ch, off] = emask_s[s:s + k]
            xr[c, ch, off, :] = x[row_s[s:s + k]]
            pos += TILE * tiles_per_block[b]
        assert pos == e_pad

    def col_layout(a):
        # [n_chunks, CHUNK] -> [n_tiles, CHUNK, NCH]  (partition-major)
        return np.ascontiguousarray(
            a.reshape(n_tiles, NCH, CHUNK).transpose(0, 2, 1))

    per_core = []
    for c in range(N_CORES):
        x_sh = np.zeros((ns_pad, 3), np.float32)
        x_sh[:ns] = x[c * ns:(c + 1) * ns]
        per_core.append({
            "col_idx": col_layout(col_idx[c]),
            "rl_col": col_layout(rl[c]),
            "em_col": col_layout(em[c]),
            "rlT": np.ascontiguousarray(rl[c].reshape(n_tiles, 1, TILE)),
            "attrT": np.ascontiguousarray(at[c].reshape(n_tiles, 1, TILE)),
            "emT": np.ascontiguousarray(em[c].reshape(n_tiles, 1, TILE)),
            "xr_col": np.ascontiguousarray(
                xr[c].reshape(n_tiles, NCH, CHUNK, 3)
                .transpose(0, 2, 1, 3).reshape(n_tiles, CHUNK, NCH * 3)),
            "x_sh": x_sh,
        })
    meta = dict(n_nodes=n_nodes, ns=ns, ns_pad=ns_pad, n_blocks=n_blocks,
                tiles_per_block=tiles_per_block, e_pad=e_pad,
                n_chunks=n_chunks, n_tiles=n_tiles)
    return per_core, meta


def make_bias_arrays(inp):
    z = np.zeros(D, np.float32)
    cols = np.stack([
        np.asarray(inp.get(k, z), np.float32).reshape(-1) for k in
        ("b_lin", "ba1", "be1", "be2", "bn1", "bn2", "bc1", "bc2")
    ], axis=1)                                    # [128, 8]
    iota_col = np.arange(NB, dtype=np.float32).reshape(NB, 1)
    cols = np.concatenate([cols, iota_col], axis=1)    # [128, 9]
    rows = np.stack([
        np.asarray(inp["ln_g"], np.float32).reshape(-1),
        np.asarray(inp["ln_b"], np.float32).reshape(-1),
        np.full(D, np.float32(np.asarray(inp["ba2"]).reshape(-1)[0])),
        np.arange(D, dtype=np.float32),
    ], axis=0)                                    # [4, 128]
    return cols, rows



# --------------------------------------------------------------- tile patches
# This container's walrus build rejects instructions carrying more than one
# semaphore wait ("Too many sync wait commands").  Redistribute excess waits
# onto single-wait InstNoOp carriers placed just before each instruction on
# the same engine (engine queues are FIFO, so gating is preserved).
_MAX_WAITS = 1
_carrier_n = [0]
_patched = [False]


def _make_carrier(mybir, engine, waits):
    nop = mybir.InstNoOp(name=f"waitcarrier_{_carrier_n[0]}", ins=[], outs=[])
    _carrier_n[0] += 1
    nop.engine = engine
    nop.sync_info = mybir.SyncInfo(on_wait=list(waits), on_update=[])
    return nop


def apply_tile_patch():
    if _patched[0]:
        return
    _patched[0] = True
    import concourse.tile as tile
    import concourse.mybir as mybir

    _orig_lower = tile.TileContext._lower_ordered_insts

    def _patched_lower(self, ordered):
        for bb_name, insts in ordered.items():
            out = []
            for inst in insts:
                si = inst.sync_info
                waits = list(si.on_wait) if si is not None and si.on_wait else []
                if len(waits) > _MAX_WAITS:
                    extra, keep = waits[:-_MAX_WAITS], waits[-_MAX_WAITS:]
                    for k in range(0, len(extra), _MAX_WAITS):
                        out.append(_make_carrier(mybir, inst.engine,
                                                 extra[k:k + _MAX_WAITS]))
                    si.on_wait = keep
                out.append(inst)
            ordered[bb_name] = out
        return _orig_lower(self, ordered)

    def _patched_drain_and_barrier(self, tick_clock, wait_clock):
        from concourse.tile import ScopedClock

        nc = self.nc
        assert self.sems is not None
        allocated = list(self.sems.allocated().values())
        carriers = []
        if allocated:
            for _ in range(48):
                carriers.append(nc.sync.wait_ge(allocated[0], 0))
        drain_inst = nc.sync.drain()
        wait_clock.add_sem_waits(
            drain_inst.ins, ScopedClock({None: tick_clock.global_clock}))
        si = drain_inst.ins.sync_info
        waits = list(si.on_wait) if si and si.on_wait else []
        if len(waits) > 1 and carriers:
            assert len(waits) <= 48, f"need more carriers: {len(waits)}"
            for c, w in zip(carriers, waits[:-1]):
                c.ins.sync_info.on_wait = [w]
            si.on_wait = [waits[-1]]
        nc.all_engine_barrier()
        popped = nc._tile_sem_poison_stack.pop()
        assert popped is self._sem_poison
        nc.clear_and_free_semaphores(allocated)
        nc.all_engine_barrier()

    tile.TileContext._lower_ordered_insts = _patched_lower
    tile.TileContext._drain_and_barrier = _patched_drain_and_barrier


# ------------------------------------------------------------------ bass build
def build_nc(meta, collectives=True):
    import concourse.bass as bass
    import concourse.tile as tile
    from concourse import mybir
    from concourse.masks import make_identity

    apply_tile_patch()

    AF = mybir.ActivationFunctionType
    OP = mybir.AluOpType
    f32 = mybir.dt.float32

    n_nodes = meta["n_nodes"]
    ns = meta["ns"]
    ns_pad = meta["ns_pad"]
    n_blocks = meta["n_blocks"]
    tiles_per_block = meta["tiles_per_block"]
    n_chunks = meta["n_chunks"]
    n_tiles = meta["n_tiles"]
    n_node_tiles = math.ceil(n_nodes / 128)

    nc = bass.Bass("TRN2", target_bir_lowering=False, debug=False,
                   num_devices=N_CORES)

    di = lambda name, shape, dt=f32: nc.dram_tensor(name, shape, dt,
                                                    kind="ExternalInput")
    h_in = di("h_full", [n_nodes, D])
    x_in = di("x_full", [n_nodes, 3])
    h_sh_in = di("h_sh", [ns_pad, D])
    W_lin = di("W_lin", [D, D])
    Wa1 = di("Wa1", [2 * D + 3, D])
    Wa2 = di("Wa2", [D, 1])
    We1 = di("We1", [D + 3, D])
    We2 = di("We2", [D, D])
    Wn1 = di("Wn1", [D, D])
    Wn2 = di("Wn2", [D, D])
    Wc1 = di("Wc1", [2 * D + 3, D])
    Wc2 = di("Wc2", [D, D])
    Wc3 = di("Wc3", [D, 1])
    bcol_in = di("bias_cols", [D, 9])
    brow_in = di("bias_rows", [4, D])
    colx_in = di("col_idx", [n_tiles, CHUNK, NCH], mybir.dt.int32)
    rlc_in = di("rl_col", [n_tiles, CHUNK, NCH])
    emc_in = di("em_col", [n_tiles, CHUNK, NCH])
    rlT_in = di("rlT", [n_tiles, 1, TILE])
    attrT_in = di("attrT", [n_tiles, 1, TILE])
    emT_in = di("emT", [n_tiles, 1, TILE])
    xrc_in = di("xr_col", [n_tiles, CHUNK, NCH * 3])
    xsh_in = di("x_sh", [ns_pad, 3])
    nmask_in = di("nmask_sh", [ns, 1])

    out_h = nc.dram_tensor("out_h", [n_nodes, D], f32, kind="ExternalOutput")
    out_x = nc.dram_tensor("out_x", [n_nodes, 3], f32, kind="ExternalOutput")

    G1 = nc.dram_tensor("G1", [n_nodes, GROW], f32)
    ag_in = nc.dram_tensor("ag_in", [ns, D], f32)
    G2 = nc.dram_tensor("G2", [n_nodes, D], f32, addr_space="Shared")
    agx_in = nc.dram_tensor("agx_in", [ns, 3], f32)
    G2x = nc.dram_tensor("G2x", [n_nodes, 3], f32, addr_space="Shared")
    e3_dram = nc.dram_tensor("e3_dram", [n_tiles, 3, TILE], f32)
    dbg_agg = nc.dram_tensor("dbg_agg", [n_blocks, 128, 128], f32)

    with tile.TileContext(nc) as tc, ExitStack() as ctx:
        singles = ctx.enter_context(tc.tile_pool(name="singles", bufs=1))
        persist = ctx.enter_context(tc.tile_pool(name="persist", bufs=1))
        ep = ctx.enter_context(tc.tile_pool(name="ep", bufs=4))
        epc = ctx.enter_context(tc.tile_pool(name="epc", bufs=2))
        gdst = ctx.enter_context(tc.tile_pool(name="gdst", bufs=8))
        idxt = ctx.enter_context(tc.tile_pool(name="idxt", bufs=8))
        npo = ctx.enter_context(tc.tile_pool(name="npo", bufs=2))
        # PSUM: exactly 8 banks
        ps_gat = ctx.enter_context(tc.tile_pool(name="ps_gat", bufs=1,
                                                space="PSUM"))   # 3 banks
        ps_mlp = ctx.enter_context(tc.tile_pool(name="ps_mlp", bufs=1,
                                                space="PSUM"))   # 2 banks
        ps_sml = ctx.enter_context(tc.tile_pool(name="ps_sml", bufs=1,
                                                space="PSUM"))   # 2 banks
        ps_agg = ctx.enter_context(tc.tile_pool(name="ps_agg", bufs=1,
                                                space="PSUM"))   # 1 bank

        def GAT_HR():
            return ps_gat.tile([128, TILE], f32, tag="gat_hr", name="gat_hr")

        def GAT_HC():
            return ps_gat.tile([128, TILE], f32, tag="gat_hc", name="gat_hc")

        def GAT_MSGT():
            return ps_gat.tile([128, TILE], f32, tag="gat_msgT", name="gat_msgT")

        def MLP_A():
            return ps_mlp.tile([128, TILE], f32, tag="mlp_A", name="mlp_A",
                               bufs=2)

        def MLP_B():
            return ps_mlp.tile([128, TILE], f32, tag="mlp_B", name="mlp_B")

        def SML_ROW():
            return ps_sml.tile([1, TILE], f32, tag="sml_row", name="sml_row")

        # ---------------- weights / constants ----------------
        _wn = [0]

        def wtile(ap, shape):
            nm = f"wt{_wn[0]}"
            _wn[0] += 1
            t = singles.tile(shape, f32, name=nm, tag=nm)
            nc.sync.dma_start(out=t[:], in_=ap[:])
            return t

        w_lin = wtile(W_lin, [D, D])
        wa1a = wtile(Wa1[0:D, :], [D, D])
        wa1b = wtile(Wa1[D:2 * D, :], [D, D])
        wa1c = wtile(Wa1[2 * D:2 * D + 3, :], [3, D])
        wa2 = wtile(Wa2, [D, 1])
        we1a = wtile(We1[0:D, :], [D, D])
        we1b = wtile(We1[D:D + 3, :], [3, D])
        we2 = wtile(We2, [D, D])
        wn1 = wtile(Wn1, [D, D])
        wn2 = wtile(Wn2, [D, D])
        wc1a = wtile(Wc1[0:D, :], [D, D])
        wc1b = wtile(Wc1[D:2 * D, :], [D, D])
        wc1c = wtile(Wc1[2 * D:2 * D + 3, :], [3, D])
        wc2 = wtile(Wc2, [D, D])
        wc3 = wtile(Wc3, [D, 1])
        bc = wtile(bcol_in, [D, 9])
        b_lin, ba1, be1, be2 = bc[:, 0:1], bc[:, 1:2], bc[:, 2:3], bc[:, 3:4]
        bn1, bn2, bc1, bc2 = bc[:, 4:5], bc[:, 5:6], bc[:, 6:7], bc[:, 7:8]
        iota_col = bc[:, 8:9]
        ln_g = wtile(brow_in[0:1, :], [1, D])
        ln_b = wtile(brow_in[1:2, :], [1, D])
        ba2_row = wtile(brow_in[2:3, :], [1, D])
        iota_row = wtile(brow_in[3:4, :], [1, D])
        ba2_s = ba2_row[0:1, 0:1]

        identity = singles.tile([128, 128], f32)
        make_identity(nc, identity[:])
        ones_col = singles.tile([128, 1], f32)
        nc.vector.memset(ones_col[:], 1.0)
        ones_row = singles.tile([1, 128], f32)
        nc.vector.memset(ones_row[:], 1.0)
        eps_col = singles.tile([128, 1], f32)
        nc.vector.memset(eps_col[:], LN_EPS)
        eps8_col = singles.tile([128, 1], f32)
        nc.vector.memset(eps8_col[:], 1e-8)

        # broadcast constants: iota / ln_g / ln_b replicated to all partitions
        iota_bc = singles.tile([128, 128], f32)
        lng_bc = singles.tile([128, 128], f32)
        lnb_bc = singles.tile([128, 128], f32)
        bc_ps = GAT_HR()
        nc.tensor.matmul(bc_ps[:, 0:128], ones_row[:], iota_row[:],
                         start=True, stop=True)
        nc.vector.tensor_copy(iota_bc[:], bc_ps[:, 0:128])
        bc_ps2 = GAT_HC()
        nc.tensor.matmul(bc_ps2[:, 0:128], ones_row[:], ln_g[:],
                         start=True, stop=True)
        nc.vector.tensor_copy(lng_bc[:], bc_ps2[:, 0:128])
        bc_ps3 = GAT_MSGT()
        nc.tensor.matmul(bc_ps3[:, 0:128], ones_row[:], ln_b[:],
                         start=True, stop=True)
        nc.vector.tensor_copy(lnb_bc[:], bc_ps3[:, 0:128])

        # persisted per-edge / per-node data
        cd_sb = persist.tile([128, n_chunks, 4], f32)            # coord_diff
        hhn_sb = persist.tile([128, n_blocks, 128], f32)         # hh_new rows
        hh_own = persist.tile([128, n_blocks, 128], f32)         # hh own rows
        x_blk = persist.tile([128, n_blocks, 4], f32)            # x own rows
        agg_sb = persist.tile([128, n_blocks, 128], f32)         # aggT

        # ============ prologue A: hh = h @ W_lin + b_lin -> G1 ============
        for i in range(n_node_tiles):
            p = min(128, n_nodes - i * 128)
            ht = ep.tile([128, D], f32, tag="ht")
            nc.sync.dma_start(out=ht[:p], in_=h_in[i * 128:i * 128 + p, :])
            hT_ps = GAT_HR()
            nc.tensor.transpose(out=hT_ps[:128, :p], in_=ht[:p, :],
                                identity=identity[:p, :p])
            hT = epc.tile([128, 128], f32, tag="w_hT")
            nc.scalar.activation(hT[:, :p], hT_ps[:, :p], AF.Copy)
            hhT_ps = MLP_A()
            nc.tensor.matmul(hhT_ps[:, :p], w_lin[:], hT[:, :p],
                             start=True, stop=True)
            hhT = epc.tile([128, 128], f32, tag="w_hhT")
            nc.scalar.activation(hhT[:, :p], hhT_ps[:, :p], AF.Identity,
                                 bias=b_lin)
            hh_ps = GAT_HC()
            nc.tensor.transpose(out=hh_ps[:p, 0:128], in_=hhT[:, :p],
                                identity=identity[:])
            gt = epc.tile([128, GROW], f32, tag="w_gt")
            nc.vector.tensor_copy(gt[:p, 0:D], hh_ps[:p, 0:128])
            xt = ep.tile([128, 4], f32, tag="xt")
            nc.sync.dma_start(out=xt[:p, 0:3], in_=x_in[i * 128:i * 128 + p, :])
            nc.vector.tensor_copy(gt[:p, D:D + 3], xt[:p, 0:3])
            nc.vector.memset(gt[:p, D + 3:GROW], 0.0)
            nc.sync.dma_start(out=G1[i * 128:i * 128 + p, :], in_=gt[:p, :])

        # ============ prologue B: hh/x for own row blocks (SBUF) ============
        for b in range(n_blocks):
            hsb = ep.tile([128, D], f32, tag="ht")
            nc.sync.dma_start(out=hsb[:], in_=h_sh_in[b * NB:(b + 1) * NB, :])
            hT_ps = GAT_HR()
            nc.tensor.transpose(out=hT_ps[:, 0:128], in_=hsb[:],
                                identity=identity[:])
            hT = epc.tile([128, 128], f32, tag="w_hT")
            nc.scalar.activation(hT[:], hT_ps[:, 0:128], AF.Copy)
            hhT_ps = MLP_A()
            nc.tensor.matmul(hhT_ps[:, 0:128], w_lin[:], hT[:],
                             start=True, stop=True)
            hhT = epc.tile([128, 128], f32, tag="w_hhT")
            nc.scalar.activation(hhT[:], hhT_ps[:, 0:128], AF.Identity,
                                 bias=b_lin)
            hh_ps = GAT_HC()
            nc.tensor.transpose(out=hh_ps[:, 0:128], in_=hhT[:],
                                identity=identity[:])
            nc.vector.tensor_copy(hh_own[:, b, :], hh_ps[:, 0:128])
            xbt = ep.tile([128, 4], f32, tag="xt")
            nc.sync.dma_start(out=xbt[:, 0:3],
                              in_=xsh_in[b * NB:(b + 1) * NB, :])
            nc.vector.tensor_copy(x_blk[:, b, 0:3], xbt[:, 0:3])

        # ================= phase 1: GCLayer edge pass =================
        def edge_tile_phase1(t, b, start, stop, agg_ps):
            c0 = t * NCH
            rlT_t = ep.tile([1, TILE], f32, tag="rlT")
            nc.sync.dma_start(out=rlT_t[:], in_=rlT_in[t])
            rlc_t = ep.tile([128, NCH], f32, tag="rlc")
            nc.sync.dma_start(out=rlc_t[:], in_=rlc_in[t])
            emc_t = ep.tile([128, NCH], f32, tag="emc")
            nc.sync.dma_start(out=emc_t[:], in_=emc_in[t])
            idx_t = idxt.tile([128, NCH], mybir.dt.int32, tag="idx")
            nc.sync.dma_start(out=idx_t[:], in_=colx_in[t])

            graw = gdst.tile([128, NCH, GROW], f32, tag="graw")
            for c in range(NCH):
                nc.gpsimd.indirect_dma_start(
                    out=graw[:, c, :], out_offset=None, in_=G1[:],
                    in_offset=bass.IndirectOffsetOnAxis(
                        ap=idx_t[:, c:c + 1], axis=0),
                )

            rlbc_sb = epc.tile([128, TILE], f32, tag="rlbc", name="rlbc_sb")
            rl_bcast_ap = bass.AP(
                tensor=rlT_in[t].tensor, offset=rlT_in[t].offset,
                ap=[[0, 128], [1, TILE]])
            nc.sync.dma_start(out=rlbc_sb[:], in_=rl_bcast_ap)
            onehotT = epc.tile([128, TILE], f32, tag="onehotT")
            nc.vector.tensor_scalar(
                onehotT[:], rlbc_sb[:], iota_col, None, OP.is_equal)

            hrT_ps = GAT_HR()
            nc.tensor.matmul(hrT_ps[:], hh_own[:, b, :], onehotT[:],
                             start=True, stop=True)
            hrT = epc.tile([128, TILE], f32, tag="hrT")
            nc.scalar.activation(hrT[:], hrT_ps[:], AF.Copy)

            hcT_ps = GAT_HC()
            for c in range(NCH):
                nc.tensor.transpose(out=hcT_ps[:, c * 128:(c + 1) * 128],
                                    in_=graw[:, c, 0:D], identity=identity[:])
            hcT = epc.tile([128, TILE], f32, tag="hcT")
            nc.vector.tensor_copy(hcT[:], hcT_ps[:])

            dT = epc.tile([128, TILE], f32, tag="dT")
            nc.vector.tensor_sub(dT[:], hcT_ps[:], hrT[:])
            sqT = epc.tile([128, TILE], f32, tag="sqT")
            nc.scalar.activation(sqT[:], dT[:], AF.Square)

            geo2_ps = SML_ROW()
            nc.tensor.matmul(geo2_ps[:], ones_col[:], sqT[:],
                             start=True, stop=True)
            e3 = epc.tile([4, TILE], f32, tag="e3cur", name="e3cur")
            georow = ep.tile([1, TILE], f32, tag="georow")
            nc.scalar.activation(georow[:], geo2_ps[:], AF.Sqrt,
                                 bias=eps8_col[0:1, :])
            nc.sync.dma_start(out=e3[2:3, :], in_=georow[:])
            nc.sync.dma_start(out=e3[1:2, :], in_=attrT_in[t])

            # coord path in [e, 3] layout, per chunk (xr host-gathered)
            xrc_t = ep.tile([128, NCH * 3], f32, tag="xrc")
            nc.sync.dma_start(out=xrc_t[:], in_=xrc_in[t])
            dist_ps = SML_ROW()
            diff = ep.tile([128, NCH, 3], f32, tag="diff")
            nc.vector.tensor_sub(
                diff[:], xrc_t[:].rearrange("p (c k) -> p c k", k=3),
                graw[:, :, D:D + 3])
            rad = ep.tile([128, NCH], f32, tag="rad")
            sqd = ep.tile([128, NCH, 3], f32, tag="sqd")
            for c in range(NCH):
                nc.scalar.activation(sqd[:, c, :], diff[:, c, :], AF.Square,
                                     accum_out=rad[:, c:c + 1])
            dist_c = ep.tile([128, NCH], f32, tag="dist")
            nc.scalar.activation(dist_c[:], rad[:], AF.Sqrt, bias=eps8_col[:])
            den = ep.tile([128, NCH], f32, tag="den")
            nc.vector.tensor_scalar_add(den[:], dist_c[:], NORM_CONST)
            rec = ep.tile([128, NCH], f32, tag="rec")
            nc.vector.reciprocal(rec[:], den[:])
            for c in range(NCH):
                nc.vector.tensor_scalar_mul(cd_sb[:, c0 + c, 0:3],
                                            diff[:, c, :], rec[:, c:c + 1])
                nc.tensor.transpose(out=dist_ps[:, c * 128:(c + 1) * 128],
                                    in_=dist_c[:, c:c + 1],
                                    identity=identity[:])
            nc.vector.tensor_copy(e3[0:1, :], dist_ps[:])
            nc.sync.dma_start(out=e3_dram[t], in_=e3[0:3, :])

            att1_ps = MLP_A()
            nc.tensor.matmul(att1_ps[:], wa1a[:], hrT[:], start=True, stop=False)
            nc.tensor.matmul(att1_ps[:], wa1b[:], hcT[:], start=False,
                             stop=False)
            nc.tensor.matmul(att1_ps[:], wa1c[:], e3[0:3, :], start=False,
                             stop=True)
            satt1 = epc.tile([128, TILE], f32, tag="satt1")
            nc.scalar.activation(satt1[:], att1_ps[:], AF.Silu, bias=ba1)
            att2_ps = SML_ROW()
            nc.tensor.matmul(att2_ps[:], wa2[:], satt1[:],
                             start=True, stop=True)
            attT = ep.tile([1, TILE], f32, tag="attT")
            nc.scalar.activation(attT[:], att2_ps[:], AF.Sigmoid,
                                 bias=ba2_s)

            msg1_ps = MLP_B()
            nc.tensor.matmul(msg1_ps[:], we1a[:], dT[:], start=True, stop=False)
            nc.tensor.matmul(msg1_ps[:], we1b[:], e3[0:3, :], start=False,
                             stop=True)
            smsg1 = epc.tile([128, TILE], f32, tag="smsg1")
            nc.scalar.activation(smsg1[:], msg1_ps[:], AF.Silu, bias=be1)
            msg2_ps = MLP_B()
            nc.tensor.matmul(msg2_ps[:], we2[:], smsg1[:], start=True,
                             stop=True)
            msgb = epc.tile([128, TILE], f32, tag="msgb")
            nc.scalar.activation(msgb[:], msg2_ps[:], AF.Identity, bias=be2)
            # fold att * edge_mask into msg in transposed space
            emT_t = ep.tile([1, TILE], f32, tag="emT")
            nc.sync.dma_start(out=emT_t[:], in_=emT_in[t])
            attm = ep.tile([1, TILE], f32, tag="attm")
            nc.vector.tensor_tensor(attm[:], attT[:], emT_t[:], OP.mult)
            attbc_ps = GAT_MSGT()
            nc.tensor.matmul(attbc_ps[:], ones_row[:], attm[:],
                             start=True, stop=True)
            msgs = epc.tile([128, TILE], f32, tag="msgs")
            nc.vector.tensor_tensor(msgs[:], msgb[:], attbc_ps[:], OP.mult)

            msgT_ps = GAT_MSGT()
            for c in range(NCH):
                nc.tensor.transpose(out=msgT_ps[:, c * 128:(c + 1) * 128],
                                    in_=msgs[:, c * 128:(c + 1) * 128],
                                    identity=identity[:])
            msg_sb = epc.tile([128, NCH, 128], f32, tag="msg_sb")
            nc.vector.tensor_copy(msg_sb[:], msgT_ps[:])
            for c in range(NCH):
                onehot = epc.tile([128, NCH, 128], f32, tag="onehot")
                nc.vector.tensor_scalar(
                    onehot[:, c, :], iota_bc[:],
                    rlc_t[:, c:c + 1], None, OP.is_equal)
                nc.tensor.matmul(agg_ps[:], msg_sb[:, c, :], onehot[:, c, :],
                                 start=(start and c == 0),
                                 stop=(stop and c == NCH - 1))

        t = 0
        for b in range(n_blocks):
            agg_ps = ps_agg.tile([128, 128], f32, tag="agg")
            for k in range(tiles_per_block[b]):
                edge_tile_phase1(t, b, start=(k == 0),
                                 stop=(k == tiles_per_block[b] - 1),
                                 agg_ps=agg_ps)
                t += 1
            nc.vector.tensor_copy(agg_sb[:, b, :], agg_ps[:])
            nc.sync.dma_start(out=dbg_agg[b], in_=agg_sb[:, b, :])
        assert t == n_tiles

        # ================= node step: MLP + LN + silu, AllGather ==========
        for b in range(n_blocks):
            nvalid = min(NB, ns - b * NB)
            z1_ps = MLP_A()
            nc.tensor.matmul(z1_ps[:, 0:128], wn1[:], agg_sb[:, b, :],
                             start=True, stop=True)
            sz1 = npo.tile([128, 128], f32, tag="sz1")
            nc.scalar.activation(sz1[:], z1_ps[:, 0:128], AF.Silu, bias=bn1)
            z2_ps = MLP_B()
            nc.tensor.matmul(z2_ps[:, 0:128], wn2[:], sz1[:], start=True,
                             stop=True)
            z2T = npo.tile([128, 128], f32, tag="z2T")
            nc.scalar.activation(z2T[:], z2_ps[:, 0:128], AF.Identity,
                                 bias=bn2)
            z2n_ps = GAT_HR()
            nc.tensor.transpose(out=z2n_ps[:, 0:128], in_=z2T[:],
                                identity=identity[:])
            s = npo.tile([128, 128], f32, tag="s")
            nc.vector.tensor_add(s[:], z2n_ps[:, 0:128], hh_own[:, b, :])
            stats = npo.tile([128, 6], f32, tag="stats")
            nc.vector.bn_stats(out=stats[:], in_=s[:])
            mv = npo.tile([128, 2], f32, tag="mv")
            nc.vector.bn_aggr(out=mv[:], in_=stats[:])
            sd = npo.tile([128, 1], f32, tag="sd")
            nc.scalar.activation(sd[:], mv[:, 1:2], AF.Sqrt, bias=eps_col)
            rstd = npo.tile([128, 1], f32, tag="rstd")
            nc.vector.reciprocal(rstd[:], sd[:])
            y = npo.tile([128, 128], f32, tag="y")
            nc.vector.tensor_scalar(y[:], s[:], mv[:, 0:1], rstd[:],
                                    OP.subtract, OP.mult)
            yg = npo.tile([128, 128], f32, tag="yg")
            nc.vector.tensor_tensor(yg[:], y[:], lng_bc[:], OP.mult)
            yb = npo.tile([128, 128], f32, tag="yb")
            nc.vector.tensor_tensor(yb[:], yg[:], lnb_bc[:], OP.add)
            nc.scalar.activation(hhn_sb[:, b, :], yb[:], AF.Silu)
            nc.sync.dma_start(out=ag_in[b * NB:b * NB + nvalid, :],
                              in_=hhn_sb[:nvalid, b, :])

        if collectives:
            with tc.tile_critical():
                cc1 = nc.alloc_semaphore("cc1")
                nc.gpsimd.collective_compute(
                    "AllGather", mybir.AluOpType.bypass,
                    ins=[ag_in[:]], outs=[G2[:]],
                    replica_groups=[list(range(N_CORES))],
                ).then_inc(cc1, 1)
                nc.gpsimd.wait_ge(cc1, 1)
        else:
            for bb in range(n_blocks):
                nv = min(NB, ns - bb * NB)
                tmpg = npo.tile([128, D], f32, tag="sz1", name="tmpg")
                nc.sync.dma_start(out=tmpg[:nv], in_=ag_in[bb * NB:bb * NB + nv, :])
                nc.sync.dma_start(out=G2[bb * NB:bb * NB + nv, :],
                                  in_=tmpg[:nv])

        for i in range(n_node_tiles):
            p = min(128, n_nodes - i * 128)
            tcp = ep.tile([128, D], f32, tag="ht")
            nc.sync.dma_start(out=tcp[:p], in_=G2[i * 128:i * 128 + p, :])
            nc.sync.dma_start(out=out_h[i * 128:i * 128 + p, :], in_=tcp[:p])

        # ================= phase 2: coord MLP edge pass ===================
        def edge_tile_phase2(t, b, start, stop, agx_ps):
            c0 = t * NCH
            rlT_t = ep.tile([1, TILE], f32, tag="rlT")
            nc.sync.dma_start(out=rlT_t[:], in_=rlT_in[t])
            rlc_t = ep.tile([128, NCH], f32, tag="rlc")
            nc.sync.dma_start(out=rlc_t[:], in_=rlc_in[t])
            emc_t = ep.tile([128, NCH], f32, tag="emc")
            nc.sync.dma_start(out=emc_t[:], in_=emc_in[t])
            idx_t = idxt.tile([128, NCH], mybir.dt.int32, tag="idx")
            nc.sync.dma_start(out=idx_t[:], in_=colx_in[t])

            graw = gdst.tile([128, NCH, D], f32, tag="graw2")
            for c in range(NCH):
                nc.gpsimd.indirect_dma_start(
                    out=graw[:, c, :], out_offset=None, in_=G2[:],
                    in_offset=bass.IndirectOffsetOnAxis(
                        ap=idx_t[:, c:c + 1], axis=0),
                )

            rlbc_sb = epc.tile([128, TILE], f32, tag="rlbc", name="rlbc_sb")
            rl_bcast_ap = bass.AP(
                tensor=rlT_in[t].tensor, offset=rlT_in[t].offset,
                ap=[[0, 128], [1, TILE]])
            nc.sync.dma_start(out=rlbc_sb[:], in_=rl_bcast_ap)
            onehotT = epc.tile([128, TILE], f32, tag="onehotT")
            nc.vector.tensor_scalar(
                onehotT[:], rlbc_sb[:], iota_col, None, OP.is_equal)

            grT_ps = GAT_HR()
            nc.tensor.matmul(grT_ps[:], hhn_sb[:, b, :], onehotT[:],
                             start=True, stop=True)
            grT = epc.tile([128, TILE], f32, tag="hrT")
            nc.scalar.activation(grT[:], grT_ps[:], AF.Copy)

            gcT_ps = GAT_HC()
            for c in range(NCH):
                nc.tensor.transpose(out=gcT_ps[:, c * 128:(c + 1) * 128],
                                    in_=graw[:, c, :], identity=identity[:])
            gcT = epc.tile([128, TILE], f32, tag="hcT")
            nc.vector.tensor_copy(gcT[:], gcT_ps[:])

            e3 = epc.tile([4, TILE], f32, tag="e3cur", name="e3cur")
            nc.sync.dma_start(out=e3[0:3, :], in_=e3_dram[t])
            m1_ps = MLP_A()
            nc.tensor.matmul(m1_ps[:], wc1a[:], grT[:], start=True, stop=False)
            nc.tensor.matmul(m1_ps[:], wc1b[:], gcT[:], start=False,
                             stop=False)
            nc.tensor.matmul(m1_ps[:], wc1c[:], e3[0:3, :], start=False,
                             stop=True)
            sm1 = epc.tile([128, TILE], f32, tag="satt1")
            nc.scalar.activation(sm1[:], m1_ps[:], AF.Silu, bias=bc1)
            m2_ps = MLP_B()
            nc.tensor.matmul(m2_ps[:], wc2[:], sm1[:], start=True, stop=True)
            sm2 = epc.tile([128, TILE], f32, tag="smsg1")
            nc.scalar.activation(sm2[:], m2_ps[:], AF.Silu, bias=bc2)
            mT_ps = SML_ROW()
            nc.tensor.matmul(mT_ps[:], wc3[:], sm2[:], start=True, stop=True)
            mrow = ep.tile([1, TILE], f32, tag="attT")
            nc.vector.tensor_copy(mrow[:], mT_ps[:])

            mc_ps = ps_gat.tile([128, TILE], f32, tag="gat_msgT",
                                name="mc_ps")
            for c in range(NCH):
                nc.tensor.transpose(out=mc_ps[:, c:c + 1],
                                    in_=mrow[:, c * 128:(c + 1) * 128],
                                    identity=identity[0:1, 0:1])
            fac = ep.tile([128, NCH], f32, tag="fac")
            nc.vector.tensor_tensor(fac[:], mc_ps[:, 0:NCH], emc_t[:],
                                    OP.mult)
            for c in range(NCH):
                trans = epc.tile([128, NCH, 4], f32, tag="trans")
                nc.vector.tensor_scalar_mul(trans[:, c, 0:3],
                                            cd_sb[:, c0 + c, 0:3],
                                            fac[:, c:c + 1])
                onehot = epc.tile([128, NCH, 128], f32, tag="onehot")
                nc.vector.tensor_scalar(
                    onehot[:, c, :], iota_bc[:],
                    rlc_t[:, c:c + 1], None, OP.is_equal)
                nc.tensor.matmul(agx_ps[0:3, :], trans[:, c, 0:3],
                                 onehot[:, c, :],
                                 start=(start and c == 0),
                                 stop=(stop and c == NCH - 1))

        t = 0
        for b in range(n_blocks):
            nvalid = min(NB, ns - b * NB)
            agx_ps = ps_agg.tile([128, 128], f32, tag="agg")
            for k in range(tiles_per_block[b]):
                edge_tile_phase2(t, b, start=(k == 0),
                                 stop=(k == tiles_per_block[b] - 1),
                                 agx_ps=agx_ps)
                t += 1
            agx_sb = npo.tile([4, 128], f32, tag="agx_sb")
            nc.vector.tensor_copy(agx_sb[0:3, :], agx_ps[0:3, :])
            agxT_ps = GAT_HR()
            nc.tensor.transpose(out=agxT_ps[:, 0:3], in_=agx_sb[0:3, :],
                                identity=identity[0:3, 0:3])
            nmt = npo.tile([128, 1], f32, tag="nmt")
            nc.sync.dma_start(out=nmt[:nvalid],
                              in_=nmask_in[b * NB:b * NB + nvalid, :])
            xo = npo.tile([128, 4], f32, tag="xo")
            nc.vector.scalar_tensor_tensor(
                xo[:, 0:3], agxT_ps[:, 0:3], 1.0 / NORM_FACTOR,
                x_blk[:, b, 0:3], OP.mult, OP.add)
            xom = npo.tile([128, 4], f32, tag="xom")
            nc.vector.tensor_scalar_mul(xom[:, 0:3], xo[:, 0:3], nmt[:])
            nc.sync.dma_start(out=agx_in[b * NB:b * NB + nvalid, :],
                              in_=xom[:nvalid, 0:3])
        assert t == n_tiles

        if collectives:
            with tc.tile_critical():
                cc2 = nc.alloc_semaphore("cc2")
                nc.gpsimd.collective_compute(
                    "AllGather", mybir.AluOpType.bypass,
                    ins=[agx_in[:]], outs=[G2x[:]],
                    replica_groups=[list(range(N_CORES))],
                ).then_inc(cc2, 1)
                nc.gpsimd.wait_ge(cc2, 1)
        else:
            tmpx = npo.tile([128, 4], f32, tag="xo", name="tmpx")
            nc.sync.dma_start(out=tmpx[:, 0:3], in_=agx_in[0:128, :])
            nc.sync.dma_start(out=G2x[0:128, :], in_=tmpx[:, 0:3])

        for i in range(n_node_tiles):
            p = min(128, n_nodes - i * 128)
            tcp = ep.tile([128, 4], f32, tag="xt")
            nc.sync.dma_start(out=tcp[:p, 0:3], in_=G2x[i * 128:i * 128 + p, :])
            nc.sync.dma_start(out=out_x[i * 128:i * 128 + p, :],
                              in_=tcp[:p, 0:3])

    return nc


# ------------------------------------------------------------------ run infra
def make_callable(nc, n_cores=N_CORES):
    import jax
    from jax.sharding import Mesh, PartitionSpec
    from jax.experimental.shard_map import shard_map
    import concourse.mybir as mybir
    from concourse import bass2jax

    bass2jax.install_neuronx_cc_hook()
    partition_name = nc.partition_id_tensor.name if nc.partition_id_tensor else None
    in_names, out_names, out_avals, zero_outs = [], [], [], []
    for alloc in nc.m.functions[0].allocations:
        if not isinstance(alloc, mybir.MemoryLocationSet):
            continue
        name = alloc.memorylocations[0].name
        if alloc.kind == "ExternalInput":
            if name != partition_name:
                in_names.append(name)
        elif alloc.kind == "ExternalOutput":
            out_names.append(name)
            out_avals.append(jax.core.ShapedArray(
                tuple(alloc.tensor_shape), mybir.dt.np(alloc.dtype)))
            zero_outs.append(np.zeros(tuple(alloc.tensor_shape),
                                      mybir.dt.np(alloc.dtype)))
    n_params = len(in_names)
    all_names = in_names + out_names + ([partition_name] if partition_name else [])

    def _body(*args):
        operands = list(args)
        if partition_name is not None:
            operands.append(bass2jax.partition_id_tensor())
        return tuple(bass2jax._bass_exec_p.bind(
            *operands, out_avals=tuple(out_avals), in_names=tuple(all_names),
            out_names=tuple(out_names), lowering_input_output_aliases=(),
            sim_require_finite=False, sim_require_nnan=False, nc=nc))

    mesh = Mesh(np.asarray(jax.devices()[:n_cores]), ("core",))
    n_outs = len(out_names)
    fn = jax.jit(
        shard_map(_body, mesh=mesh,
                  in_specs=(PartitionSpec("core"),) * (n_params + n_outs),
                  out_specs=(PartitionSpec("core"),) * n_outs,
                  check_rep=False),
        keep_unused=True)
    return fn, in_names, out_names, zero_outs, mesh


def prep_in_maps(inputs):
    n_nodes = np.asarray(inputs["h"]).shape[0]
    per_core, meta = host_prep(
        inputs["x"], inputs["edge_index"], inputs["edge_mask"],
        inputs["edge_attr"], n_nodes)
    bcols, brows = make_bias_arrays(inputs)
    h = np.ascontiguousarray(np.asarray(inputs["h"], np.float32))
    x = np.ascontiguousarray(np.asarray(inputs["x"], np.float32))
    nmask = np.asarray(inputs["node_mask"], np.float32).reshape(-1, 1)
    ns, ns_pad = meta["ns"], meta["ns_pad"]
    shared = {
        "h_full": h, "x_full": x,
        "bias_cols": bcols, "bias_rows": brows,
    }
    for k in ("W_lin", "Wa1", "Wa2", "We1", "We2", "Wn1", "Wn2", "Wc1",
              "Wc2", "Wc3"):
        shared[k] = np.ascontiguousarray(np.asarray(inputs[k], np.float32))
    in_maps = []
    for c in range(N_CORES):
        m = dict(shared)
        m.update(per_core[c])
        h_sh = np.zeros((ns_pad, D), np.float32)
        h_sh[:ns] = h[c * ns:(c + 1) * ns]
        m["h_sh"] = h_sh
        m["nmask_sh"] = np.ascontiguousarray(nmask[c * ns:(c + 1) * ns])
        in_maps.append(m)
    return in_maps, meta


def kernel(**inputs):
    in_maps, meta = prep_in_maps(inputs)
    key = (meta["e_pad"], tuple(meta["tiles_per_block"]), meta["n_nodes"])
    if key not in _CACHE:
        nc = build_nc(meta)
        _CACHE[key] = (nc,) + make_callable(nc)
    nc, fn, in_names, out_names, zero_outs, mesh = _CACHE[key]

    import jax
    from jax.sharding import NamedSharding, PartitionSpec
    sh = NamedSharding(mesh, PartitionSpec("core"))
    big_in = [
        jax.device_put(
            np.ascontiguousarray(np.concatenate(
                [np.asarray(in_maps[c][n]) for c in range(N_CORES)], axis=0)),
            sh)
        for n in in_names
    ]
    big_zeros = [
        jax.device_put(np.zeros((N_CORES * z.shape[0], *z.shape[1:]), z.dtype),
                       sh)
        for z in zero_outs
    ]
    outs = fn(*big_in, *big_zeros)
    jax.block_until_ready(outs)
    res = {name: np.asarray(outs[i]).reshape(N_CORES, *zero_outs[i].shape)[0]
           for i, name in enumerate(out_names)}
    _CACHE["last_run"] = (fn, big_in, big_zeros, out_names, zero_outs)
    return res["out_h"], res["out_x"]


def rerun_timed(n_reps=20):
    import jax
    fn, big_in, big_zeros, out_names, zero_outs = _CACHE["last_run"]
    ts = []
    for _ in range(n_reps):
        t0 = time.perf_counter()
        outs = fn(*big_in, *big_zeros)
        jax.block_until_ready(outs)
        ts.append(time.perf_counter() - t0)
    return np.array(ts)
